# revision 5
# baseline (speedup 1.0000x reference)
"""Longformer attention Bass kernel for 8 TRN2 NeuronCores (v4).

Problem: B=2, H=16, N=2048, D=64, window=256, global positions 0..3.
Sharding: B*H = 32 heads -> 4 heads per core (head-parallel).

v4 changes over v3 (48.5us):
  - Zero-init PV matmuls removed: PV pieces are rescheduled so each O^T
    PSUM bank's FIRST writer is the full-bank piece of chunk 4k+1
    (start=True zeroes the whole 2KB bank); earlier chunks' pieces for
    that bank are deferred into that phase.  -3.4us of PE.
  - exp split between ACT and DVE: per uniform phase the ACT engine
    exps ring[:, 0:XA]; the DVE computes ring[:, XA:1280] with a fused
    Schraudolph bitcast-exp: one scalar_tensor_tensor
      i32 = int32(s * A + Bpat)   (store-convert does the 2^x trick)
    where Bpat is B_VALID on in-band entries and C_MASK on masked ones
    (masked entries land at ~2^-100: the band mask rides for free), and
    the bf16 P^T for the PV matmul is the upper 16 bits of each int32
    read through a stride-2 bitcast view (bf16 round via +2^15 in B).
    Schraudolph rel err ~1.8% RMS on the offloaded columns.
  - mask multiplies for the ACT part moved to GPSIMD (Pool), freeing
    the DVE; dead mc0 mask removed.
  - O^T bank copies split ACT/DVE by knob.
"""

import numpy as np
import ml_dtypes

B, H, N, D = 2, 16, 2048, 64
W = 256
NG = 4  # global positions 0..3
NCORES = 8
HPC = (B * H) // NCORES  # heads per core = 4
NKC = N // 128  # key chunks = 16
BF16 = ml_dtypes.bfloat16

# ---- tuning knobs ----------------------------------------------------------
XA = 640  # uniform-phase cols [0:XA) exp'd on ACT; [XA:1280) Schraudolph on DVE
SCHR_DELTA = 0.0575  # Schraudolph centering (fraction of 2^23)
A_CONST = float(np.float32((1 << 23) / np.log(2.0)))
B_VALID = float(np.float32((127 << 23) + 32768 - SCHR_DELTA * (1 << 23)))
C_MASK = float(np.float32(1 << 27))
# engine for bf16 mask multiplies on the ACT part: 'g' (Pool) or 'v' (DVE)
MASK_ENG = "g"
# (h, bank) pairs whose O^T copy runs on ACT instead of DVE
COPY_ON_ACT = {(3, 2)}

# phase grouping of key chunks
PHASES = [[0, 1], [2, 3], [4, 5], [6, 7], [8, 9], [10, 11], [12, 13], [14, 15]]


def phases_for(h):
    return PHASES


# ot bank b is complete after the PV pass of this phase index
BANK_DONE_PHASE = {0: 2, 1: 4, 2: 6, 3: 7}


def bank_done_for(h):
    return BANK_DONE_PHASE


def chunk_qs(kc: int) -> int:
    if kc == 0:
        return 0
    return min(max(128 * kc - W, 0), N - 384)


def chunk_width(kc: int) -> int:
    if kc in (1, 14):
        return 512
    if kc in (0, 15):
        return 384
    return 640


def chunk_masks(kc: int):
    """Mask ops for chunk kc in chunk-local columns: (col0, col1, mask)."""
    if kc == 0:
        return [(256, 384, "mtrail")]
    if kc == 1:
        return [(384, 512, "mtrail")]
    if kc in (14, 15):
        return [(0, 128, "mlead")]
    return [(0, 128, "mlead"), (512, 640, "mtrail")]


def pv_pieces(kc: int):
    """PV output piece spans for chunk kc: (abs_col0, abs_col1)."""
    qs, w = chunk_qs(kc), chunk_width(kc)
    pieces = []
    a = qs
    while a < qs + w:
        b = min((a // 512 + 1) * 512, qs + w)
        pieces.append((a, b))
        a = b
    return pieces


def pv_schedule():
    """{phase: [(kc, a, b, is_full_bank_start)]} with each bank's first
    writer being the full-bank piece of chunk 4k+1 (start=True zeroes the
    bank); earlier chunks' pieces for that bank are deferred to that phase."""
    sched = {pi: [] for pi in range(8)}
    for kc in range(NKC):
        cp = kc // 2
        for a, b in pv_pieces(kc):
            bank = a // 512
            full = kc == 4 * bank + 1 and a == 512 * bank and b == a + 512
            p = cp if kc >= 4 * bank + 1 else 2 * bank
            sched[p].append((kc, a, b, full))
    for p in sched:
        sched[p].sort(key=lambda t: (not t[3], t[0], t[1]))
    return sched


PV_SCHED = pv_schedule()

# uniform-phase mask regions in absolute phase columns
UNI_REGIONS = [(0, 128, "mlead"), (512, 640, "mtrail"),
               (640, 768, "mlead"), (1152, 1280, "mtrail")]


def phase_cols(pi, h=0):
    """[(kc, col_off, width)] within the phase tile."""
    off = 0
    out = []
    for kc in phases_for(h)[pi]:
        w = chunk_width(kc)
        out.append((kc, off, w))
        off += w
    return out


def phase_uniform(pi):
    return all(chunk_width(kc) == 640 for kc in PHASES[pi])


# ---------------------------------------------------------------------------
# Numpy model of the exact device algorithm (geometry validation)
# ---------------------------------------------------------------------------


def _mask_tiles_np():
    p = np.arange(128)[:, None]
    j = np.arange(128)[None, :]
    return {
        "mlead": (j >= p).astype(np.float32),
        "mtrail": (j <= p).astype(np.float32),
    }


def _bpat_np():
    """[128, 1280-XA] f32 Schraudolph bias pattern for uniform phases."""
    bp = np.full((128, 1280 - XA), np.float32(B_VALID), np.float32)
    masks = _mask_tiles_np()
    for r0, r1, mname in UNI_REGIONS:
        s0, s1 = max(r0, XA), r1
        if s0 >= s1:
            continue
        p = np.arange(128)[:, None]
        j = np.arange(s0 - r0, s1 - r0)[None, :]
        keep = (j >= p) if mname == "mlead" else (j <= p)
        bp[:, s0 - XA : s1 - XA] = np.where(keep, np.float32(B_VALID),
                                            np.float32(C_MASK))
    return bp


def _schraudolph_np(st, bpat):
    y = st.astype(np.float32) * np.float32(A_CONST) + bpat
    i = y.astype(np.int32)
    return ((i.view(np.uint32) >> np.uint32(16)).astype(np.uint16)
            .view(BF16).astype(np.float32))


def numpy_model_head(qT, kT, vx):
    """qT/kT: [64, N] bf16-rounded f32 (q pre-scaled); vx: [N, 65] bf16-rounded.

    Returns OT [65, N] f32 (unnormalized band-only O^T + denominator row).
    """
    qT = qT.astype(np.float32)
    kT = kT.astype(np.float32)
    vx = vx.astype(np.float32)
    masks = _mask_tiles_np()
    bpat = _bpat_np()
    ot = np.zeros((65, N), np.float32)
    for pi in range(8):
        cols = phase_cols(pi)
        wtot = cols[-1][1] + cols[-1][2]
        st = np.empty((128, wtot), np.float32)
        for kc, off, w in cols:
            qs = chunk_qs(kc)
            kk = slice(128 * kc, 128 * kc + 128)
            st[:, off : off + w] = kT[:, kk].T @ qT[:, qs : qs + w]
        if phase_uniform(pi):
            pt = np.empty((128, wtot), np.float32)
            pt[:, :XA] = np.exp(st[:, :XA]).astype(BF16).astype(np.float32)
            pt[:, XA:] = _schraudolph_np(st[:, XA:], bpat)
            for r0, r1, mname in UNI_REGIONS:
                e = min(r1, XA)
                if r0 < e:
                    pt[:, r0:e] *= masks[mname][:, : e - r0]
        else:
            pt = np.exp(st).astype(BF16).astype(np.float32)
            for kc, off, w in cols:
                for c0, c1, mname in chunk_masks(kc):
                    pt[:, off + c0 : off + c1] *= masks[mname][:, : c1 - c0]
        pt = pt.astype(BF16).astype(np.float32)
        for kc, off, w in cols:
            qs = chunk_qs(kc)
            kk = slice(128 * kc, 128 * kc + 128)
            ot[:, qs : qs + w] += vx[kk].T @ pt[:, off : off + w]
    return ot


# ---------------------------------------------------------------------------
# Host-side prep / unprep
# ---------------------------------------------------------------------------


def prep_core_inputs(Q, K, V, core):
    """Q/K/V: [B*H, N, D] f32. Returns the in_map for one core."""
    h0 = core * HPC
    qt = np.empty((2, 128, N), BF16)
    kt = np.empty((2, 128, N), BF16)
    vx = np.zeros((HPC, 128, NKC, 65), BF16)
    for p in range(2):
        for s in range(2):
            h = h0 + 2 * p + s
            qt[p, 64 * s : 64 * s + 64] = (Q[h].T * np.float32(0.125)).astype(BF16)
            kt[p, 64 * s : 64 * s + 64] = K[h].T.astype(BF16)
    for i in range(HPC):
        v = np.concatenate([V[h0 + i], np.ones((N, 1), np.float32)], axis=1)
        vx[i] = v.reshape(NKC, 128, 65).transpose(1, 0, 2).astype(BF16)
    return {"qt": qt, "kt": kt, "vx": vx}


def host_glob_strips(Q, K, V):
    """f32 contributions of the 4 global KEYS beyond the window (k < q-256)."""
    scale = np.float32(0.125)
    s = np.einsum("hqd,hkd->hqk", Q[:, 256:].astype(np.float32), K[:, 0:NG]) * scale
    e = np.exp(s)  # [BH, N-256, NG]
    q_abs = np.arange(256, N)[None, :, None]
    k_idx = np.arange(NG)[None, None, :]
    e = e * (k_idx < q_abs - 256)
    gnum = np.einsum("hqk,hkd->hqd", e, V[:, 0:NG])
    gden = e.sum(axis=-1)
    return gnum, gden


def host_global_rows(Q, K, V):
    """Exact f32 attention for the 4 global query rows of every head."""
    scale = np.float32(1.0 / np.sqrt(D))
    s = np.einsum("hqd,hkd->hqk", Q[:, :NG].astype(np.float32), K) * scale
    s -= s.max(axis=-1, keepdims=True)
    p = np.exp(s)
    p /= p.sum(axis=-1, keepdims=True)
    return np.einsum("hqk,hkd->hqd", p, V)


def unprep_output(ot_all, og, gnum, gden):
    out = np.empty((B * H, N, D), np.float32)
    for core in range(NCORES):
        ot = np.asarray(ot_all[core])
        for i in range(HPC):
            h = core * HPC + i
            num = ot[i, :D, :].T.copy()  # [N, D]
            den = ot[i, D, :].copy()  # [N]
            num[256:] += gnum[h]
            den[256:] += gden[h]
            out[h] = num / den[:, None]
    out[:, 0:NG] = og
    return out.reshape(B, H, N, D)


# ---------------------------------------------------------------------------
# Bass module
# ---------------------------------------------------------------------------

_CACHED_NC = None


def build_module():
    global _CACHED_NC
    if _CACHED_NC is not None:
        return _CACHED_NC
    from contextlib import ExitStack

    import concourse.bass as bass  # noqa: F401
    import concourse.tile as tile
    from concourse import bacc, mybir

    f32 = mybir.dt.float32
    bf16 = mybir.dt.bfloat16
    i32 = mybir.dt.int32
    EXP = mybir.ActivationFunctionType.Exp
    GE = mybir.AluOpType.is_ge
    MULT = mybir.AluOpType.mult
    ADD = mybir.AluOpType.add

    nc = bacc.Bacc("TRN2", target_bir_lowering=False, debug=False)
    qt_d = nc.dram_tensor("qt", [2, 128, N], bf16, kind="ExternalInput")
    kt_d = nc.dram_tensor("kt", [2, 128, N], bf16, kind="ExternalInput")
    vx_d = nc.dram_tensor("vx", [HPC, 128, NKC, 65], bf16, kind="ExternalInput")
    ot_d = nc.dram_tensor("ot", [HPC, 65, N], f32, kind="ExternalOutput")

    with tile.TileContext(nc) as tc, ExitStack() as ctx:
        io_pool = ctx.enter_context(tc.tile_pool(name="io", bufs=1))
        msk_pool = ctx.enter_context(tc.tile_pool(name="msk", bufs=1))
        pt_pool = ctx.enter_context(tc.tile_pool(name="ptp", bufs=4))
        pti_pool = ctx.enter_context(tc.tile_pool(name="pti", bufs=4))
        osb_pool = ctx.enter_context(tc.tile_pool(name="osb", bufs=3))
        ring_pool = ctx.enter_context(tc.tile_pool(name="ring", bufs=1, space="PSUM"))
        po_pool = ctx.enter_context(tc.tile_pool(name="po", bufs=2, space="PSUM"))

        # ---- static PSUM: two 3-bank score rings ----
        ringA = ring_pool.tile([128, 1536], f32, tag="ringA", name="ringA")
        ringB = ring_pool.tile([128, 1536], f32, tag="ringB", name="ringB")
        rings = [ringA, ringB]

        # ---- inputs ----
        qt_sb = []
        kt_sb = []
        vx_sb = []
        for pair in range(2):
            qtp = io_pool.tile([128, N], bf16, tag=f"qt{pair}", name=f"qt{pair}")
            ktp = io_pool.tile([128, N], bf16, tag=f"kt{pair}", name=f"kt{pair}")
            qt_sb.append(qtp)
            kt_sb.append(ktp)
        for h in range(HPC):
            vxh = io_pool.tile([128, NKC, 65], bf16, tag=f"vx{h}", name=f"vx{h}")
            vx_sb.append(vxh)
        # issue order == transfer order; pair-0 Q/K lead pieces gate the
        # first QKs, everything else streams
        nc.sync.dma_start(out=kt_sb[0][:, 0:768], in_=kt_d[0][:, 0:768])
        nc.scalar.dma_start(out=qt_sb[0][:, 0:1152], in_=qt_d[0][:, 0:1152])
        nc.sync.dma_start(out=qt_sb[0][:, 1152:N], in_=qt_d[0][:, 1152:N])
        nc.scalar.dma_start(out=vx_sb[0][:], in_=vx_d[0])
        nc.sync.dma_start(out=kt_sb[1][:], in_=kt_d[1])
        nc.scalar.dma_start(out=qt_sb[1][:], in_=qt_d[1])
        nc.sync.dma_start(out=vx_sb[3][:], in_=vx_d[3])

        # ---- warm the PE pstate while the first DMAs land ----
        wu = msk_pool.tile([64, 512], bf16, tag="wu", name="wu")
        nc.vector.memset(wu[:, 0:128], 0.0)
        nc.tensor.matmul(
            ringB[:, 0:128], wu[:, 0:128], wu[:, 0:128],
            start=True, stop=True, skip_group_check=True,
        )
        nc.vector.memset(wu[:, 128:512], 0.0)
        for i in range(5):
            nc.tensor.matmul(
                ringB[:, 0:512], wu[:, 0:128], wu[:],
                start=True, stop=True, skip_group_check=True,
            )

        def qh(h):
            return qt_sb[h // 2][64 * (h % 2) : 64 * (h % 2) + 64, :]

        def kh(h):
            return kt_sb[h // 2][64 * (h % 2) : 64 * (h % 2) + 64, :]

        # ---- mask tiles (0/1 bf16) + Schraudolph bias pattern (f32) ----
        mlead2 = msk_pool.tile([128, 2, 128], bf16, tag="mlead2", name="mlead2")
        mtrail2 = msk_pool.tile([128, 2, 128], bf16, tag="mtrail2", name="mtrail2")
        nc.gpsimd.memset(mlead2[:], 1.0)
        nc.gpsimd.memset(mtrail2[:], 1.0)
        nc.gpsimd.affine_select(
            mlead2[:], mlead2[:], pattern=[[0, 2], [1, 128]], base=0,
            channel_multiplier=-1, compare_op=GE, fill=0.0,
        )
        nc.gpsimd.affine_select(
            mtrail2[:], mtrail2[:], pattern=[[0, 2], [-1, 128]], base=0,
            channel_multiplier=1, compare_op=GE, fill=0.0,
        )
        MASKS = {"mlead": mlead2, "mtrail": mtrail2}

        bpat = msk_pool.tile([128, 1280 - XA], f32, tag="bpat", name="bpat")
        nc.gpsimd.memset(bpat[:], B_VALID)
        for r0, r1, mname in UNI_REGIONS:
            s0 = max(r0, XA)
            if s0 >= r1:
                continue
            w = r1 - s0
            shift = s0 - r0
            if mname == "mlead":  # keep (j_local + shift) - p >= 0
                nc.gpsimd.affine_select(
                    bpat[:, s0 - XA : r1 - XA], bpat[:, s0 - XA : r1 - XA],
                    pattern=[[1, w]], base=shift,
                    channel_multiplier=-1, compare_op=GE, fill=C_MASK,
                )
            else:  # keep p - (j_local + shift) >= 0
                nc.gpsimd.affine_select(
                    bpat[:, s0 - XA : r1 - XA], bpat[:, s0 - XA : r1 - XA],
                    pattern=[[-1, w]], base=-shift,
                    channel_multiplier=1, compare_op=GE, fill=C_MASK,
                )

        # late-needed inputs on the SWDGE queue after mask gen
        nc.gpsimd.dma_start(out=kt_sb[0][:, 768:N], in_=kt_d[0][:, 768:N])
        nc.gpsimd.dma_start(out=vx_sb[1][:], in_=vx_d[1])
        nc.gpsimd.dma_start(out=vx_sb[2][:], in_=vx_d[2])

        # ---- per-(head, phase) emitters ----
        pt_tiles = {}

        def emit_qk(u, h, pi):
            ring = rings[u % 2]
            for kc, off, w in phase_cols(pi, h):
                klhs = kh(h)[:, 128 * kc : 128 * kc + 128]
                qs = chunk_qs(kc)
                a = 0
                while a < w:  # split at ring bank boundaries
                    b = min(((off + a) // 512 + 1) * 512 - off, w)
                    nc.tensor.matmul(
                        ring[:, off + a : off + b],
                        klhs,
                        qh(h)[:, qs + a : qs + b],
                        start=True, stop=True, skip_group_check=True,
                    )
                    a = b

        def emit_exp(u, h, pi):
            ring = rings[u % 2]
            cols = phase_cols(pi, h)
            wtot = cols[-1][1] + cols[-1][2]
            if phase_uniform(pi):
                pa = pt_pool.tile([128, XA], bf16, tag="pt", name=f"pt_h{h}p{pi}")
                nc.scalar.activation(pa[:, :], ring[:, 0:XA], EXP)
                pd = pti_pool.tile(
                    [128, 1280 - XA], i32, tag="pti", name=f"pti_h{h}p{pi}"
                )
                nc.vector.scalar_tensor_tensor(
                    out=pd[:], in0=ring[:, XA:1280], scalar=A_CONST,
                    in1=bpat[:], op0=MULT, op1=ADD,
                )
                pt_tiles[(h, pi)] = (pa, pd)
            else:
                pa = pt_pool.tile([128, wtot], bf16, tag="pt", name=f"pt_h{h}p{pi}")
                nc.scalar.activation(pa[:, 0:wtot], ring[:, 0:wtot], EXP)
                pt_tiles[(h, pi)] = (pa, None)

        def ptslice(h, pi, kc, c0, c1):
            """bf16 AP for chunk-local cols [c0,c1) of chunk kc in phase pi."""
            pa, pd = pt_tiles[(h, pi)]
            off = dict((k, o) for k, o, _ in phase_cols(pi, h))[kc]
            a0, a1 = off + c0, off + c1
            if pd is None or a1 <= XA:
                return pa[:, a0:a1]
            assert a0 >= XA, f"piece straddles XA: {a0}..{a1}"
            return (
                pd[:, a0 - XA : a1 - XA]
                .bitcast(bf16)
                .rearrange("p (n two) -> p n two", two=2)[:, :, 1]
            )

        def mask_eng():
            return nc.gpsimd if MASK_ENG == "g" else nc.vector

        def emit_masks(h, pi):
            pa, pd = pt_tiles[(h, pi)]
            if phase_uniform(pi):
                for r0, r1, mname in UNI_REGIONS:
                    e = min(r1, XA)
                    if r0 >= e:
                        continue
                    m = MASKS[mname]
                    mask_eng().tensor_mul(
                        pa[:, r0:e], pa[:, r0:e], m[:, 0, 0 : e - r0]
                    )
                return
            for kc, off, w in phase_cols(pi, h):
                for c0, c1, mname in chunk_masks(kc):
                    m = MASKS[mname]
                    mask_eng().tensor_mul(
                        pa[:, off + c0 : off + c1],
                        pa[:, off + c0 : off + c1],
                        m[:, 0, 0 : c1 - c0],
                    )

        bank_tiles = {}
        otq = [nc.sync, nc.scalar, nc.sync, nc.gpsimd]

        def emit_pv(h, pi):
            for kc, a, b, full in PV_SCHED[pi]:
                cpi = kc // 2  # the chunk's own phase (deferred pieces: < pi)
                bank = a // 512
                key = (h, bank)
                if full:
                    assert key not in bank_tiles
                    bank_tiles[key] = po_pool.tile(
                        [65, 512], f32, tag="ot", name=f"ot_h{h}b{bank}"
                    )
                bt = bank_tiles[key]
                vstat = vx_sb[h][:, kc, :]
                qs = chunk_qs(kc)
                # split at the XA boundary inside uniform phases so each
                # matmul's pt slice is wholly ACT-side or DVE-side
                splits = [(a, b)]
                if phase_uniform(cpi):
                    off = dict((k, o) for k, o, _ in phase_cols(cpi, h))[kc]
                    qx = qs + (XA - off)  # q at the XA boundary
                    if a < qx < b:
                        splits = [(a, qx), (qx, b)]
                for s0, s1 in splits:
                    nc.tensor.matmul(
                        bt[:, s0 - 512 * bank : s1 - 512 * bank],
                        vstat,
                        ptslice(h, cpi, kc, s0 - qs, s1 - qs),
                        start=full and s0 == a, stop=False,
                        skip_group_check=True,
                    )
            done = [bank for bank, dpi in bank_done_for(h).items() if dpi == pi]
            if done:
                bank = done[0]
                bt = bank_tiles.pop((h, bank))
                osb = osb_pool.tile(
                    [65, 512], f32, tag="osb", name=f"osb_h{h}b{bank}"
                )
                if (h, bank) in COPY_ON_ACT:
                    nc.scalar.copy(out=osb[:], in_=bt[:])
                else:
                    nc.vector.tensor_copy(out=osb[:], in_=bt[:])
                q = otq[(4 * h + bank + h) % len(otq)]
                q.dma_start(
                    out=ot_d[h][:, 512 * bank : 512 * bank + 512], in_=osb[:]
                )

        # ---- software-pipelined emission: PV lags QK by 2 units ----
        units = [(h, pi) for h in range(HPC) for pi in range(8)]
        L = len(units)
        for u, (h, pi) in enumerate(units):
            emit_qk(u, h, pi)
            emit_exp(u, h, pi)
            emit_masks(h, pi)
            if u >= 2:
                emit_pv(*units[u - 2])
            if u >= 3:
                pt_tiles.pop(units[u - 3])
        emit_pv(*units[L - 2])
        pt_tiles.pop(units[L - 3])
        emit_pv(*units[L - 1])
        pt_tiles.pop(units[L - 2])
        pt_tiles.pop(units[L - 1])

    nc.compile()
    _CACHED_NC = nc
    return nc


# ---------------------------------------------------------------------------
# Entry points
# ---------------------------------------------------------------------------


def run(inputs, trace=False, trace_kwargs=None):
    """Returns (output [B,H,N,D] f32, BassKernelResults)."""
    from concourse import bass_utils

    Q = np.asarray(inputs["Q"], np.float32).reshape(B * H, N, D)
    K = np.asarray(inputs["K"], np.float32).reshape(B * H, N, D)
    V = np.asarray(inputs["V"], np.float32).reshape(B * H, N, D)
    in_maps = [prep_core_inputs(Q, K, V, c) for c in range(NCORES)]
    nc = build_module()
    res = bass_utils.run_bass_kernel_spmd(
        nc,
        in_maps,
        core_ids=list(range(NCORES)),
        trace=trace,
        **(trace_kwargs or {}),
    )
    ot_all = [res.results[c]["ot"] for c in range(NCORES)]
    og = host_global_rows(Q, K, V)
    gnum, gden = host_glob_strips(Q, K, V)
    return unprep_output(ot_all, og, gnum, gden), res


def kernel(**inputs) -> np.ndarray:
    out, _ = run(inputs, trace=False)
    return out


# revision 7
# speedup vs baseline: 1.0179x; 1.0179x over previous
"""Longformer attention Bass kernel for 8 TRN2 NeuronCores (v4).

Problem: B=2, H=16, N=2048, D=64, window=256, global positions 0..3.
Sharding: B*H = 32 heads -> 4 heads per core (head-parallel).

v4 changes over v3 (48.5us):
  - Zero-init PV matmuls removed: PV pieces are rescheduled so each O^T
    PSUM bank's FIRST writer is the full-bank piece of chunk 4k+1
    (start=True zeroes the whole 2KB bank); earlier chunks' pieces for
    that bank are deferred into that phase.  -3.4us of PE.
  - exp split between ACT and DVE: per uniform phase the ACT engine
    exps ring[:, 0:XA]; the DVE computes ring[:, XA:1280] with a fused
    Schraudolph bitcast-exp: one scalar_tensor_tensor
      i32 = int32(s * A + Bpat)   (store-convert does the 2^x trick)
    where Bpat is B_VALID on in-band entries and C_MASK on masked ones
    (masked entries land at ~2^-100: the band mask rides for free), and
    the bf16 P^T for the PV matmul is the upper 16 bits of each int32
    read through a stride-2 bitcast view (bf16 round via +2^15 in B).
    Schraudolph rel err ~1.8% RMS on the offloaded columns.
  - mask multiplies for the ACT part moved to GPSIMD (Pool), freeing
    the DVE; dead mc0 mask removed.
  - O^T bank copies split ACT/DVE by knob.
"""

import numpy as np
import ml_dtypes

B, H, N, D = 2, 16, 2048, 64
W = 256
NG = 4  # global positions 0..3
NCORES = 8
HPC = (B * H) // NCORES  # heads per core = 4
NKC = N // 128  # key chunks = 16
BF16 = ml_dtypes.bfloat16

# ---- tuning knobs ----------------------------------------------------------
XA = 640  # uniform-phase cols [0:XA) exp'd on ACT; [XA:1280) Schraudolph on DVE
SCHR_DELTA = 0.0575  # Schraudolph centering (fraction of 2^23)
A_CONST = float(np.float32((1 << 23) / np.log(2.0)))
B_VALID = float(np.float32((127 << 23) + 32768 - SCHR_DELTA * (1 << 23)))
C_MASK = float(np.float32(1 << 27))
# engine for bf16 mask multiplies on the ACT part: 'g' (Pool) or 'v' (DVE)
MASK_ENG = "g"
# (h, bank) pairs whose O^T copy runs on ACT instead of DVE: spread them so
# neither engine's in-order queue delays the ring-WAR critical chain
COPY_ON_ACT = {(0, 1), (1, 0), (1, 3), (2, 2), (3, 1), (3, 2)}

# phase grouping of key chunks
PHASES = [[0, 1], [2, 3], [4, 5], [6, 7], [8, 9], [10, 11], [12, 13], [14, 15]]


def phases_for(h):
    return PHASES


# ot bank b is complete after the PV pass of this phase index
BANK_DONE_PHASE = {0: 2, 1: 4, 2: 6, 3: 7}


def bank_done_for(h):
    return BANK_DONE_PHASE


def chunk_qs(kc: int) -> int:
    if kc == 0:
        return 0
    return min(max(128 * kc - W, 0), N - 384)


def chunk_width(kc: int) -> int:
    if kc in (1, 14):
        return 512
    if kc in (0, 15):
        return 384
    return 640


def chunk_masks(kc: int):
    """Mask ops for chunk kc in chunk-local columns: (col0, col1, mask)."""
    if kc == 0:
        return [(256, 384, "mtrail")]
    if kc == 1:
        return [(384, 512, "mtrail")]
    if kc in (14, 15):
        return [(0, 128, "mlead")]
    return [(0, 128, "mlead"), (512, 640, "mtrail")]


def pv_pieces(kc: int):
    """PV output piece spans for chunk kc: (abs_col0, abs_col1)."""
    qs, w = chunk_qs(kc), chunk_width(kc)
    pieces = []
    a = qs
    while a < qs + w:
        b = min((a // 512 + 1) * 512, qs + w)
        pieces.append((a, b))
        a = b
    return pieces


def pv_schedule():
    """{phase: [(kc, a, b, is_full_bank_start)]} with each bank's first
    writer being the full-bank piece of chunk 4k+1 (start=True zeroes the
    bank); earlier chunks' pieces for that bank are deferred to that phase."""
    sched = {pi: [] for pi in range(8)}
    for kc in range(NKC):
        cp = kc // 2
        for a, b in pv_pieces(kc):
            bank = a // 512
            full = kc == 4 * bank + 1 and a == 512 * bank and b == a + 512
            p = cp if kc >= 4 * bank + 1 else 2 * bank
            sched[p].append((kc, a, b, full))
    for p in sched:
        sched[p].sort(key=lambda t: (not t[3], t[0], t[1]))
    return sched


PV_SCHED = pv_schedule()

# uniform-phase mask regions in absolute phase columns
UNI_REGIONS = [(0, 128, "mlead"), (512, 640, "mtrail"),
               (640, 768, "mlead"), (1152, 1280, "mtrail")]


def phase_cols(pi, h=0):
    """[(kc, col_off, width)] within the phase tile."""
    off = 0
    out = []
    for kc in phases_for(h)[pi]:
        w = chunk_width(kc)
        out.append((kc, off, w))
        off += w
    return out


def phase_uniform(pi):
    return all(chunk_width(kc) == 640 for kc in PHASES[pi])


# ---------------------------------------------------------------------------
# Numpy model of the exact device algorithm (geometry validation)
# ---------------------------------------------------------------------------


def _mask_tiles_np():
    p = np.arange(128)[:, None]
    j = np.arange(128)[None, :]
    return {
        "mlead": (j >= p).astype(np.float32),
        "mtrail": (j <= p).astype(np.float32),
    }


def _bpat_np():
    """[128, 1280-XA] f32 Schraudolph bias pattern for uniform phases."""
    bp = np.full((128, 1280 - XA), np.float32(B_VALID), np.float32)
    masks = _mask_tiles_np()
    for r0, r1, mname in UNI_REGIONS:
        s0, s1 = max(r0, XA), r1
        if s0 >= s1:
            continue
        p = np.arange(128)[:, None]
        j = np.arange(s0 - r0, s1 - r0)[None, :]
        keep = (j >= p) if mname == "mlead" else (j <= p)
        bp[:, s0 - XA : s1 - XA] = np.where(keep, np.float32(B_VALID),
                                            np.float32(C_MASK))
    return bp


def _schraudolph_np(st, bpat):
    y = st.astype(np.float32) * np.float32(A_CONST) + bpat
    i = y.astype(np.int32)
    return ((i.view(np.uint32) >> np.uint32(16)).astype(np.uint16)
            .view(BF16).astype(np.float32))


def numpy_model_head(qT, kT, vx):
    """qT/kT: [64, N] bf16-rounded f32 (q pre-scaled); vx: [N, 65] bf16-rounded.

    Returns OT [65, N] f32 (unnormalized band-only O^T + denominator row).
    """
    qT = qT.astype(np.float32)
    kT = kT.astype(np.float32)
    vx = vx.astype(np.float32)
    masks = _mask_tiles_np()
    bpat = _bpat_np()
    ot = np.zeros((65, N), np.float32)
    for pi in range(8):
        cols = phase_cols(pi)
        wtot = cols[-1][1] + cols[-1][2]
        st = np.empty((128, wtot), np.float32)
        for kc, off, w in cols:
            qs = chunk_qs(kc)
            kk = slice(128 * kc, 128 * kc + 128)
            st[:, off : off + w] = kT[:, kk].T @ qT[:, qs : qs + w]
        if phase_uniform(pi):
            pt = np.empty((128, wtot), np.float32)
            pt[:, :XA] = np.exp(st[:, :XA]).astype(BF16).astype(np.float32)
            pt[:, XA:] = _schraudolph_np(st[:, XA:], bpat)
            for r0, r1, mname in UNI_REGIONS:
                e = min(r1, XA)
                if r0 < e:
                    pt[:, r0:e] *= masks[mname][:, : e - r0]
        else:
            pt = np.exp(st).astype(BF16).astype(np.float32)
            for kc, off, w in cols:
                for c0, c1, mname in chunk_masks(kc):
                    pt[:, off + c0 : off + c1] *= masks[mname][:, : c1 - c0]
        pt = pt.astype(BF16).astype(np.float32)
        for kc, off, w in cols:
            qs = chunk_qs(kc)
            kk = slice(128 * kc, 128 * kc + 128)
            ot[:, qs : qs + w] += vx[kk].T @ pt[:, off : off + w]
    return ot


# ---------------------------------------------------------------------------
# Host-side prep / unprep
# ---------------------------------------------------------------------------


def prep_core_inputs(Q, K, V, core):
    """Q/K/V: [B*H, N, D] f32. Returns the in_map for one core."""
    h0 = core * HPC
    qt = np.empty((2, 128, N), BF16)
    kt = np.empty((2, 128, N), BF16)
    vx = np.zeros((HPC, 128, NKC, 65), BF16)
    for p in range(2):
        for s in range(2):
            h = h0 + 2 * p + s
            qt[p, 64 * s : 64 * s + 64] = (Q[h].T * np.float32(0.125)).astype(BF16)
            kt[p, 64 * s : 64 * s + 64] = K[h].T.astype(BF16)
    for i in range(HPC):
        v = np.concatenate([V[h0 + i], np.ones((N, 1), np.float32)], axis=1)
        vx[i] = v.reshape(NKC, 128, 65).transpose(1, 0, 2).astype(BF16)
    return {"qt": qt, "kt": kt, "vx": vx}


def host_glob_strips(Q, K, V):
    """f32 contributions of the 4 global KEYS beyond the window (k < q-256)."""
    scale = np.float32(0.125)
    s = np.einsum("hqd,hkd->hqk", Q[:, 256:].astype(np.float32), K[:, 0:NG]) * scale
    e = np.exp(s)  # [BH, N-256, NG]
    q_abs = np.arange(256, N)[None, :, None]
    k_idx = np.arange(NG)[None, None, :]
    e = e * (k_idx < q_abs - 256)
    gnum = np.einsum("hqk,hkd->hqd", e, V[:, 0:NG])
    gden = e.sum(axis=-1)
    return gnum, gden


def host_global_rows(Q, K, V):
    """Exact f32 attention for the 4 global query rows of every head."""
    scale = np.float32(1.0 / np.sqrt(D))
    s = np.einsum("hqd,hkd->hqk", Q[:, :NG].astype(np.float32), K) * scale
    s -= s.max(axis=-1, keepdims=True)
    p = np.exp(s)
    p /= p.sum(axis=-1, keepdims=True)
    return np.einsum("hqk,hkd->hqd", p, V)


def unprep_output(ot_all, og, gnum, gden):
    out = np.empty((B * H, N, D), np.float32)
    for core in range(NCORES):
        ot = np.asarray(ot_all[core])
        for i in range(HPC):
            h = core * HPC + i
            num = ot[i, :D, :].T.copy()  # [N, D]
            den = ot[i, D, :].copy()  # [N]
            num[256:] += gnum[h]
            den[256:] += gden[h]
            out[h] = num / den[:, None]
    out[:, 0:NG] = og
    return out.reshape(B, H, N, D)


# ---------------------------------------------------------------------------
# Bass module
# ---------------------------------------------------------------------------

_CACHED_NC = None


def build_module():
    global _CACHED_NC
    if _CACHED_NC is not None:
        return _CACHED_NC
    from contextlib import ExitStack

    import concourse.bass as bass  # noqa: F401
    import concourse.tile as tile
    from concourse import bacc, mybir

    f32 = mybir.dt.float32
    bf16 = mybir.dt.bfloat16
    i32 = mybir.dt.int32
    EXP = mybir.ActivationFunctionType.Exp
    GE = mybir.AluOpType.is_ge
    MULT = mybir.AluOpType.mult
    ADD = mybir.AluOpType.add

    nc = bacc.Bacc("TRN2", target_bir_lowering=False, debug=False)
    qt_d = nc.dram_tensor("qt", [2, 128, N], bf16, kind="ExternalInput")
    kt_d = nc.dram_tensor("kt", [2, 128, N], bf16, kind="ExternalInput")
    vx_d = nc.dram_tensor("vx", [HPC, 128, NKC, 65], bf16, kind="ExternalInput")
    ot_d = nc.dram_tensor("ot", [HPC, 65, N], f32, kind="ExternalOutput")

    with tile.TileContext(nc) as tc, ExitStack() as ctx:
        io_pool = ctx.enter_context(tc.tile_pool(name="io", bufs=1))
        msk_pool = ctx.enter_context(tc.tile_pool(name="msk", bufs=1))
        pt_pool = ctx.enter_context(tc.tile_pool(name="ptp", bufs=4))
        pti_pool = ctx.enter_context(tc.tile_pool(name="pti", bufs=4))
        osb_pool = ctx.enter_context(tc.tile_pool(name="osb", bufs=3))
        ring_pool = ctx.enter_context(tc.tile_pool(name="ring", bufs=1, space="PSUM"))
        po_pool = ctx.enter_context(tc.tile_pool(name="po", bufs=2, space="PSUM"))

        # ---- static PSUM: two 3-bank score rings ----
        ringA = ring_pool.tile([128, 1536], f32, tag="ringA", name="ringA")
        ringB = ring_pool.tile([128, 1536], f32, tag="ringB", name="ringB")
        rings = [ringA, ringB]

        # ---- inputs ----
        qt_sb = []
        kt_sb = []
        vx_sb = []
        for pair in range(2):
            qtp = io_pool.tile([128, N], bf16, tag=f"qt{pair}", name=f"qt{pair}")
            ktp = io_pool.tile([128, N], bf16, tag=f"kt{pair}", name=f"kt{pair}")
            qt_sb.append(qtp)
            kt_sb.append(ktp)
        for h in range(HPC):
            vxh = io_pool.tile([128, NKC, 65], bf16, tag=f"vx{h}", name=f"vx{h}")
            vx_sb.append(vxh)
        # issue order == transfer order; pair-0 Q/K lead pieces gate the
        # first QKs, everything else streams
        nc.sync.dma_start(out=kt_sb[0][:, 0:768], in_=kt_d[0][:, 0:768])
        nc.scalar.dma_start(out=qt_sb[0][:, 0:1152], in_=qt_d[0][:, 0:1152])
        nc.sync.dma_start(out=qt_sb[0][:, 1152:N], in_=qt_d[0][:, 1152:N])
        nc.scalar.dma_start(out=vx_sb[0][:], in_=vx_d[0])
        nc.sync.dma_start(out=kt_sb[1][:], in_=kt_d[1])
        nc.scalar.dma_start(out=qt_sb[1][:], in_=qt_d[1])
        nc.sync.dma_start(out=vx_sb[3][:], in_=vx_d[3])

        # ---- warm the PE pstate while the first DMAs land ----
        wu = msk_pool.tile([64, 512], bf16, tag="wu", name="wu")
        nc.vector.memset(wu[:, 0:128], 0.0)
        nc.tensor.matmul(
            ringB[:, 0:128], wu[:, 0:128], wu[:, 0:128],
            start=True, stop=True, skip_group_check=True,
        )
        nc.vector.memset(wu[:, 128:512], 0.0)
        for i in range(5):
            nc.tensor.matmul(
                ringB[:, 0:512], wu[:, 0:128], wu[:],
                start=True, stop=True, skip_group_check=True,
            )

        def qh(h):
            return qt_sb[h // 2][64 * (h % 2) : 64 * (h % 2) + 64, :]

        def kh(h):
            return kt_sb[h // 2][64 * (h % 2) : 64 * (h % 2) + 64, :]

        # ---- mask tiles (0/1 bf16) + Schraudolph bias pattern (f32) ----
        mlead2 = msk_pool.tile([128, 2, 128], bf16, tag="mlead2", name="mlead2")
        mtrail2 = msk_pool.tile([128, 2, 128], bf16, tag="mtrail2", name="mtrail2")
        nc.gpsimd.memset(mlead2[:], 1.0)
        nc.gpsimd.memset(mtrail2[:], 1.0)
        nc.gpsimd.affine_select(
            mlead2[:], mlead2[:], pattern=[[0, 2], [1, 128]], base=0,
            channel_multiplier=-1, compare_op=GE, fill=0.0,
        )
        nc.gpsimd.affine_select(
            mtrail2[:], mtrail2[:], pattern=[[0, 2], [-1, 128]], base=0,
            channel_multiplier=1, compare_op=GE, fill=0.0,
        )
        MASKS = {"mlead": mlead2, "mtrail": mtrail2}

        bpat = msk_pool.tile([128, 1280 - XA], f32, tag="bpat", name="bpat")
        nc.gpsimd.memset(bpat[:], B_VALID)
        for r0, r1, mname in UNI_REGIONS:
            s0 = max(r0, XA)
            if s0 >= r1:
                continue
            w = r1 - s0
            shift = s0 - r0
            if mname == "mlead":  # keep (j_local + shift) - p >= 0
                nc.gpsimd.affine_select(
                    bpat[:, s0 - XA : r1 - XA], bpat[:, s0 - XA : r1 - XA],
                    pattern=[[1, w]], base=shift,
                    channel_multiplier=-1, compare_op=GE, fill=C_MASK,
                )
            else:  # keep p - (j_local + shift) >= 0
                nc.gpsimd.affine_select(
                    bpat[:, s0 - XA : r1 - XA], bpat[:, s0 - XA : r1 - XA],
                    pattern=[[-1, w]], base=-shift,
                    channel_multiplier=1, compare_op=GE, fill=C_MASK,
                )

        # late-needed inputs on the SWDGE queue after mask gen
        nc.gpsimd.dma_start(out=kt_sb[0][:, 768:N], in_=kt_d[0][:, 768:N])
        nc.gpsimd.dma_start(out=vx_sb[1][:], in_=vx_d[1])
        nc.gpsimd.dma_start(out=vx_sb[2][:], in_=vx_d[2])

        # ---- per-(head, phase) emitters ----
        pt_tiles = {}

        def emit_qk(u, h, pi):
            ring = rings[u % 2]
            for kc, off, w in phase_cols(pi, h):
                klhs = kh(h)[:, 128 * kc : 128 * kc + 128]
                qs = chunk_qs(kc)
                a = 0
                while a < w:  # split at ring bank boundaries
                    b = min(((off + a) // 512 + 1) * 512 - off, w)
                    nc.tensor.matmul(
                        ring[:, off + a : off + b],
                        klhs,
                        qh(h)[:, qs + a : qs + b],
                        start=True, stop=True, skip_group_check=True,
                    )
                    a = b

        def emit_exp(u, h, pi):
            ring = rings[u % 2]
            cols = phase_cols(pi, h)
            wtot = cols[-1][1] + cols[-1][2]
            if phase_uniform(pi):
                pa = pt_pool.tile([128, XA], bf16, tag="pt", name=f"pt_h{h}p{pi}")
                nc.scalar.activation(pa[:, :], ring[:, 0:XA], EXP)
                pd = pti_pool.tile(
                    [128, 1280 - XA], i32, tag="pti", name=f"pti_h{h}p{pi}"
                )
                nc.vector.scalar_tensor_tensor(
                    out=pd[:], in0=ring[:, XA:1280], scalar=A_CONST,
                    in1=bpat[:], op0=MULT, op1=ADD,
                )
                pt_tiles[(h, pi)] = (pa, pd)
            else:
                pa = pt_pool.tile([128, wtot], bf16, tag="pt", name=f"pt_h{h}p{pi}")
                nc.scalar.activation(pa[:, 0:wtot], ring[:, 0:wtot], EXP)
                pt_tiles[(h, pi)] = (pa, None)

        def ptslice(h, pi, kc, c0, c1):
            """bf16 AP for chunk-local cols [c0,c1) of chunk kc in phase pi."""
            pa, pd = pt_tiles[(h, pi)]
            off = dict((k, o) for k, o, _ in phase_cols(pi, h))[kc]
            a0, a1 = off + c0, off + c1
            if pd is None or a1 <= XA:
                return pa[:, a0:a1]
            assert a0 >= XA, f"piece straddles XA: {a0}..{a1}"
            return (
                pd[:, a0 - XA : a1 - XA]
                .bitcast(bf16)
                .rearrange("p (n two) -> p n two", two=2)[:, :, 1]
            )

        def mask_eng():
            return nc.gpsimd if MASK_ENG == "g" else nc.vector

        def emit_masks(h, pi):
            pa, pd = pt_tiles[(h, pi)]
            if phase_uniform(pi):
                for r0, r1, mname in UNI_REGIONS:
                    e = min(r1, XA)
                    if r0 >= e:
                        continue
                    m = MASKS[mname]
                    mask_eng().tensor_mul(
                        pa[:, r0:e], pa[:, r0:e], m[:, 0, 0 : e - r0]
                    )
                return
            for kc, off, w in phase_cols(pi, h):
                for c0, c1, mname in chunk_masks(kc):
                    m = MASKS[mname]
                    mask_eng().tensor_mul(
                        pa[:, off + c0 : off + c1],
                        pa[:, off + c0 : off + c1],
                        m[:, 0, 0 : c1 - c0],
                    )

        bank_tiles = {}
        otq = [nc.sync]

        def emit_pv(h, pi):
            for kc, a, b, full in PV_SCHED[pi]:
                cpi = kc // 2  # the chunk's own phase (deferred pieces: < pi)
                bank = a // 512
                key = (h, bank)
                if full:
                    assert key not in bank_tiles
                    bank_tiles[key] = po_pool.tile(
                        [65, 512], f32, tag="ot", name=f"ot_h{h}b{bank}"
                    )
                bt = bank_tiles[key]
                vstat = vx_sb[h][:, kc, :]
                qs = chunk_qs(kc)
                # split at the XA boundary inside uniform phases so each
                # matmul's pt slice is wholly ACT-side or DVE-side
                splits = [(a, b)]
                if phase_uniform(cpi):
                    off = dict((k, o) for k, o, _ in phase_cols(cpi, h))[kc]
                    qx = qs + (XA - off)  # q at the XA boundary
                    if a < qx < b:
                        splits = [(a, qx), (qx, b)]
                for s0, s1 in splits:
                    nc.tensor.matmul(
                        bt[:, s0 - 512 * bank : s1 - 512 * bank],
                        vstat,
                        ptslice(h, cpi, kc, s0 - qs, s1 - qs),
                        start=full and s0 == a, stop=False,
                        skip_group_check=True,
                    )
            done = [bank for bank, dpi in bank_done_for(h).items() if dpi == pi]
            if done:
                bank = done[0]
                bt = bank_tiles.pop((h, bank))
                osb = osb_pool.tile(
                    [65, 512], f32, tag="osb", name=f"osb_h{h}b{bank}"
                )
                if (h, bank) in COPY_ON_ACT:
                    nc.scalar.copy(out=osb[:], in_=bt[:])
                else:
                    nc.vector.tensor_copy(out=osb[:], in_=bt[:])
                q = otq[(4 * h + bank + h) % len(otq)]
                q.dma_start(
                    out=ot_d[h][:, 512 * bank : 512 * bank + 512], in_=osb[:]
                )

        # ---- software-pipelined emission: PV lags QK by 2 units ----
        units = [(h, pi) for h in range(HPC) for pi in range(8)]
        L = len(units)
        for u, (h, pi) in enumerate(units):
            emit_qk(u, h, pi)
            emit_exp(u, h, pi)
            emit_masks(h, pi)
            if u >= 2:
                emit_pv(*units[u - 2])
            if u >= 3:
                pt_tiles.pop(units[u - 3])
        emit_pv(*units[L - 2])
        pt_tiles.pop(units[L - 3])
        emit_pv(*units[L - 1])
        pt_tiles.pop(units[L - 2])
        pt_tiles.pop(units[L - 1])

    nc.compile()
    _CACHED_NC = nc
    return nc


# ---------------------------------------------------------------------------
# Entry points
# ---------------------------------------------------------------------------


def run(inputs, trace=False, trace_kwargs=None):
    """Returns (output [B,H,N,D] f32, BassKernelResults)."""
    from concourse import bass_utils

    Q = np.asarray(inputs["Q"], np.float32).reshape(B * H, N, D)
    K = np.asarray(inputs["K"], np.float32).reshape(B * H, N, D)
    V = np.asarray(inputs["V"], np.float32).reshape(B * H, N, D)
    in_maps = [prep_core_inputs(Q, K, V, c) for c in range(NCORES)]
    nc = build_module()
    res = bass_utils.run_bass_kernel_spmd(
        nc,
        in_maps,
        core_ids=list(range(NCORES)),
        trace=trace,
        **(trace_kwargs or {}),
    )
    ot_all = [res.results[c]["ot"] for c in range(NCORES)]
    og = host_global_rows(Q, K, V)
    gnum, gden = host_glob_strips(Q, K, V)
    return unprep_output(ot_all, og, gnum, gden), res


def kernel(**inputs) -> np.ndarray:
    out, _ = run(inputs, trace=False)
    return out


# revision 11
# speedup vs baseline: 1.0303x; 1.0121x over previous
"""Longformer attention Bass kernel for 8 TRN2 NeuronCores (v4).

Problem: B=2, H=16, N=2048, D=64, window=256, global positions 0..3.
Sharding: B*H = 32 heads -> 4 heads per core (head-parallel).

v4 changes over v3 (48.5us):
  - Zero-init PV matmuls removed: PV pieces are rescheduled so each O^T
    PSUM bank's FIRST writer is the full-bank piece of chunk 4k+1
    (start=True zeroes the whole 2KB bank); earlier chunks' pieces for
    that bank are deferred into that phase.  -3.4us of PE.
  - exp split between ACT and DVE: per uniform phase the ACT engine
    exps ring[:, 0:XA]; the DVE computes ring[:, XA:1280] with a fused
    Schraudolph bitcast-exp: one scalar_tensor_tensor
      i32 = int32(s * A + Bpat)   (store-convert does the 2^x trick)
    where Bpat is B_VALID on in-band entries and C_MASK on masked ones
    (masked entries land at ~2^-100: the band mask rides for free), and
    the bf16 P^T for the PV matmul is the upper 16 bits of each int32
    read through a stride-2 bitcast view (bf16 round via +2^15 in B).
    Schraudolph rel err ~1.8% RMS on the offloaded columns.
  - mask multiplies for the ACT part moved to GPSIMD (Pool), freeing
    the DVE; dead mc0 mask removed.
  - O^T bank copies split ACT/DVE by knob.
"""

import numpy as np
import ml_dtypes

B, H, N, D = 2, 16, 2048, 64
W = 256
NG = 4  # global positions 0..3
NCORES = 8
HPC = (B * H) // NCORES  # heads per core = 4
NKC = N // 128  # key chunks = 16
BF16 = ml_dtypes.bfloat16

# ---- tuning knobs ----------------------------------------------------------
XA = 640  # uniform-phase cols [0:XA) exp'd on ACT; [XA:1280) Schraudolph on DVE
SCHR_DELTA = 0.0575  # Schraudolph centering (fraction of 2^23)
A_CONST = float(np.float32((1 << 23) / np.log(2.0)))
B_VALID = float(np.float32((127 << 23) + 32768 - SCHR_DELTA * (1 << 23)))
C_MASK = float(np.float32(1 << 27))
# engine for bf16 mask multiplies on the ACT part: 'g' (Pool) or 'v' (DVE)
MASK_ENG = "g"
# (h, bank) pairs whose O^T copy runs on ACT instead of DVE: spread them so
# neither engine's in-order queue delays the ring-WAR critical chain
COPY_ON_ACT = {(0, 1), (1, 0), (1, 3), (2, 2), (3, 1), (3, 2)}
# PV emission lag in units: larger lag gives the QK->exp->mask->PV chain
# more pipeline slack before it gates the in-order PE program
PV_LAG = 3

# phase grouping of key chunks
PHASES = [[0, 1], [2, 3], [4, 5], [6, 7], [8, 9], [10, 11], [12, 13], [14, 15]]


def phases_for(h):
    return PHASES


# ot bank b is complete after the PV pass of this phase index
BANK_DONE_PHASE = {0: 2, 1: 4, 2: 6, 3: 7}


def bank_done_for(h):
    return BANK_DONE_PHASE


def chunk_qs(kc: int) -> int:
    if kc == 0:
        return 0
    return min(max(128 * kc - W, 0), N - 384)


def chunk_width(kc: int) -> int:
    if kc in (1, 14):
        return 512
    if kc in (0, 15):
        return 384
    return 640


def chunk_masks(kc: int):
    """Mask ops for chunk kc in chunk-local columns: (col0, col1, mask)."""
    if kc == 0:
        return [(256, 384, "mtrail")]
    if kc == 1:
        return [(384, 512, "mtrail")]
    if kc in (14, 15):
        return [(0, 128, "mlead")]
    return [(0, 128, "mlead"), (512, 640, "mtrail")]


def pv_pieces(kc: int):
    """PV output piece spans for chunk kc: (abs_col0, abs_col1)."""
    qs, w = chunk_qs(kc), chunk_width(kc)
    pieces = []
    a = qs
    while a < qs + w:
        b = min((a // 512 + 1) * 512, qs + w)
        pieces.append((a, b))
        a = b
    return pieces


def pv_schedule():
    """{phase: [(kc, a, b, is_full_bank_start)]} with each bank's first
    writer being the full-bank piece of chunk 4k+1 (start=True zeroes the
    bank); earlier chunks' pieces for that bank are deferred to that phase."""
    sched = {pi: [] for pi in range(8)}
    for kc in range(NKC):
        cp = kc // 2
        for a, b in pv_pieces(kc):
            bank = a // 512
            full = kc == 4 * bank + 1 and a == 512 * bank and b == a + 512
            p = cp if kc >= 4 * bank + 1 else 2 * bank
            sched[p].append((kc, a, b, full))
    for p in sched:
        sched[p].sort(key=lambda t: (not t[3], t[0], t[1]))
    return sched


PV_SCHED = pv_schedule()

# uniform-phase mask regions in absolute phase columns
UNI_REGIONS = [(0, 128, "mlead"), (512, 640, "mtrail"),
               (640, 768, "mlead"), (1152, 1280, "mtrail")]


def phase_cols(pi, h=0):
    """[(kc, col_off, width)] within the phase tile."""
    off = 0
    out = []
    for kc in phases_for(h)[pi]:
        w = chunk_width(kc)
        out.append((kc, off, w))
        off += w
    return out


def phase_uniform(pi):
    return all(chunk_width(kc) == 640 for kc in PHASES[pi])


# ---------------------------------------------------------------------------
# Numpy model of the exact device algorithm (geometry validation)
# ---------------------------------------------------------------------------


def _mask_tiles_np():
    p = np.arange(128)[:, None]
    j = np.arange(128)[None, :]
    return {
        "mlead": (j >= p).astype(np.float32),
        "mtrail": (j <= p).astype(np.float32),
    }


def _bpat_np():
    """[128, 1280-XA] f32 Schraudolph bias pattern for uniform phases."""
    bp = np.full((128, 1280 - XA), np.float32(B_VALID), np.float32)
    masks = _mask_tiles_np()
    for r0, r1, mname in UNI_REGIONS:
        s0, s1 = max(r0, XA), r1
        if s0 >= s1:
            continue
        p = np.arange(128)[:, None]
        j = np.arange(s0 - r0, s1 - r0)[None, :]
        keep = (j >= p) if mname == "mlead" else (j <= p)
        bp[:, s0 - XA : s1 - XA] = np.where(keep, np.float32(B_VALID),
                                            np.float32(C_MASK))
    return bp


def _schraudolph_np(st, bpat):
    y = st.astype(np.float32) * np.float32(A_CONST) + bpat
    i = y.astype(np.int32)
    return ((i.view(np.uint32) >> np.uint32(16)).astype(np.uint16)
            .view(BF16).astype(np.float32))


def numpy_model_head(qT, kT, vx):
    """qT/kT: [64, N] bf16-rounded f32 (q pre-scaled); vx: [N, 65] bf16-rounded.

    Returns OT [65, N] f32 (unnormalized band-only O^T + denominator row).
    """
    qT = qT.astype(np.float32)
    kT = kT.astype(np.float32)
    vx = vx.astype(np.float32)
    masks = _mask_tiles_np()
    bpat = _bpat_np()
    ot = np.zeros((65, N), np.float32)
    for pi in range(8):
        cols = phase_cols(pi)
        wtot = cols[-1][1] + cols[-1][2]
        st = np.empty((128, wtot), np.float32)
        for kc, off, w in cols:
            qs = chunk_qs(kc)
            kk = slice(128 * kc, 128 * kc + 128)
            st[:, off : off + w] = kT[:, kk].T @ qT[:, qs : qs + w]
        if phase_uniform(pi):
            pt = np.empty((128, wtot), np.float32)
            pt[:, :XA] = np.exp(st[:, :XA]).astype(BF16).astype(np.float32)
            pt[:, XA:] = _schraudolph_np(st[:, XA:], bpat)
            for r0, r1, mname in UNI_REGIONS:
                e = min(r1, XA)
                if r0 < e:
                    pt[:, r0:e] *= masks[mname][:, : e - r0]
        else:
            pt = np.exp(st).astype(BF16).astype(np.float32)
            for kc, off, w in cols:
                for c0, c1, mname in chunk_masks(kc):
                    pt[:, off + c0 : off + c1] *= masks[mname][:, : c1 - c0]
        pt = pt.astype(BF16).astype(np.float32)
        for kc, off, w in cols:
            qs = chunk_qs(kc)
            kk = slice(128 * kc, 128 * kc + 128)
            ot[:, qs : qs + w] += vx[kk].T @ pt[:, off : off + w]
    return ot


# ---------------------------------------------------------------------------
# Host-side prep / unprep
# ---------------------------------------------------------------------------


def prep_core_inputs(Q, K, V, core):
    """Q/K/V: [B*H, N, D] f32. Returns the in_map for one core."""
    h0 = core * HPC
    qt = np.empty((2, 128, N), BF16)
    kt = np.empty((2, 128, N), BF16)
    vx = np.zeros((HPC, 128, NKC, 65), BF16)
    for p in range(2):
        for s in range(2):
            h = h0 + 2 * p + s
            qt[p, 64 * s : 64 * s + 64] = (Q[h].T * np.float32(0.125)).astype(BF16)
            kt[p, 64 * s : 64 * s + 64] = K[h].T.astype(BF16)
    for i in range(HPC):
        v = np.concatenate([V[h0 + i], np.ones((N, 1), np.float32)], axis=1)
        vx[i] = v.reshape(NKC, 128, 65).transpose(1, 0, 2).astype(BF16)
    return {"qt": qt, "kt": kt, "vx": vx}


def host_glob_strips(Q, K, V):
    """f32 contributions of the 4 global KEYS beyond the window (k < q-256)."""
    scale = np.float32(0.125)
    s = np.einsum("hqd,hkd->hqk", Q[:, 256:].astype(np.float32), K[:, 0:NG]) * scale
    e = np.exp(s)  # [BH, N-256, NG]
    q_abs = np.arange(256, N)[None, :, None]
    k_idx = np.arange(NG)[None, None, :]
    e = e * (k_idx < q_abs - 256)
    gnum = np.einsum("hqk,hkd->hqd", e, V[:, 0:NG])
    gden = e.sum(axis=-1)
    return gnum, gden


def host_global_rows(Q, K, V):
    """Exact f32 attention for the 4 global query rows of every head."""
    scale = np.float32(1.0 / np.sqrt(D))
    s = np.einsum("hqd,hkd->hqk", Q[:, :NG].astype(np.float32), K) * scale
    s -= s.max(axis=-1, keepdims=True)
    p = np.exp(s)
    p /= p.sum(axis=-1, keepdims=True)
    return np.einsum("hqk,hkd->hqd", p, V)


def unprep_output(ot_all, og, gnum, gden):
    out = np.empty((B * H, N, D), np.float32)
    for core in range(NCORES):
        ot = np.asarray(ot_all[core])
        for i in range(HPC):
            h = core * HPC + i
            num = ot[i, :D, :].T.copy()  # [N, D]
            den = ot[i, D, :].copy()  # [N]
            num[256:] += gnum[h]
            den[256:] += gden[h]
            out[h] = num / den[:, None]
    out[:, 0:NG] = og
    return out.reshape(B, H, N, D)


# ---------------------------------------------------------------------------
# Bass module
# ---------------------------------------------------------------------------

_CACHED_NC = None


def build_module():
    global _CACHED_NC
    if _CACHED_NC is not None:
        return _CACHED_NC
    from contextlib import ExitStack

    import concourse.bass as bass  # noqa: F401
    import concourse.tile as tile
    from concourse import bacc, mybir

    f32 = mybir.dt.float32
    bf16 = mybir.dt.bfloat16
    i32 = mybir.dt.int32
    EXP = mybir.ActivationFunctionType.Exp
    GE = mybir.AluOpType.is_ge
    MULT = mybir.AluOpType.mult
    ADD = mybir.AluOpType.add

    nc = bacc.Bacc("TRN2", target_bir_lowering=False, debug=False)
    qt_d = nc.dram_tensor("qt", [2, 128, N], bf16, kind="ExternalInput")
    kt_d = nc.dram_tensor("kt", [2, 128, N], bf16, kind="ExternalInput")
    vx_d = nc.dram_tensor("vx", [HPC, 128, NKC, 65], bf16, kind="ExternalInput")
    ot_d = nc.dram_tensor("ot", [HPC, 65, N], f32, kind="ExternalOutput")

    with tile.TileContext(nc) as tc, ExitStack() as ctx:
        io_pool = ctx.enter_context(tc.tile_pool(name="io", bufs=1))
        msk_pool = ctx.enter_context(tc.tile_pool(name="msk", bufs=1))
        pt_pool = ctx.enter_context(tc.tile_pool(name="ptp", bufs=PV_LAG + 2))
        pti_pool = ctx.enter_context(tc.tile_pool(name="pti", bufs=PV_LAG + 2))
        osb_pool = ctx.enter_context(tc.tile_pool(name="osb", bufs=3))
        ring_pool = ctx.enter_context(tc.tile_pool(name="ring", bufs=1, space="PSUM"))
        po_pool = ctx.enter_context(tc.tile_pool(name="po", bufs=2, space="PSUM"))

        # ---- static PSUM: two 3-bank score rings ----
        ringA = ring_pool.tile([128, 1536], f32, tag="ringA", name="ringA")
        ringB = ring_pool.tile([128, 1536], f32, tag="ringB", name="ringB")
        rings = [ringA, ringB]

        # ---- inputs ----
        qt_sb = []
        kt_sb = []
        vx_sb = []
        for pair in range(2):
            qtp = io_pool.tile([128, N], bf16, tag=f"qt{pair}", name=f"qt{pair}")
            ktp = io_pool.tile([128, N], bf16, tag=f"kt{pair}", name=f"kt{pair}")
            qt_sb.append(qtp)
            kt_sb.append(ktp)
        for h in range(HPC):
            vxh = io_pool.tile([128, NKC, 65], bf16, tag=f"vx{h}", name=f"vx{h}")
            vx_sb.append(vxh)
        # issue order == transfer order; tiny pair-0 lead pieces unblock the
        # first QK phase as early as possible, everything else streams
        nc.sync.dma_start(out=kt_sb[0][:, 0:256], in_=kt_d[0][:, 0:256])
        nc.scalar.dma_start(out=qt_sb[0][:, 0:512], in_=qt_d[0][:, 0:512])
        nc.sync.dma_start(out=kt_sb[0][:, 256:768], in_=kt_d[0][:, 256:768])
        nc.scalar.dma_start(out=qt_sb[0][:, 512:1152], in_=qt_d[0][:, 512:1152])
        nc.sync.dma_start(out=qt_sb[0][:, 1152:N], in_=qt_d[0][:, 1152:N])
        nc.scalar.dma_start(out=vx_sb[0][:], in_=vx_d[0])
        nc.sync.dma_start(out=kt_sb[1][:], in_=kt_d[1])
        nc.scalar.dma_start(out=qt_sb[1][:], in_=qt_d[1])
        nc.sync.dma_start(out=vx_sb[3][:], in_=vx_d[3])

        # ---- warm the PE pstate while the first DMAs land ----
        wu = msk_pool.tile([64, 512], bf16, tag="wu", name="wu")
        nc.vector.memset(wu[:, 0:128], 0.0)
        nc.tensor.matmul(
            ringB[:, 0:128], wu[:, 0:128], wu[:, 0:128],
            start=True, stop=True, skip_group_check=True,
        )
        nc.vector.memset(wu[:, 128:512], 0.0)
        for i in range(5):
            nc.tensor.matmul(
                ringB[:, 0:512], wu[:, 0:128], wu[:],
                start=True, stop=True, skip_group_check=True,
            )

        def qh(h):
            return qt_sb[h // 2][64 * (h % 2) : 64 * (h % 2) + 64, :]

        def kh(h):
            return kt_sb[h // 2][64 * (h % 2) : 64 * (h % 2) + 64, :]

        # ---- mask tiles (0/1 bf16) + Schraudolph bias pattern (f32) ----
        mlead2 = msk_pool.tile([128, 2, 128], bf16, tag="mlead2", name="mlead2")
        mtrail2 = msk_pool.tile([128, 2, 128], bf16, tag="mtrail2", name="mtrail2")
        nc.gpsimd.memset(mlead2[:], 1.0)
        nc.gpsimd.memset(mtrail2[:], 1.0)
        nc.gpsimd.affine_select(
            mlead2[:], mlead2[:], pattern=[[0, 2], [1, 128]], base=0,
            channel_multiplier=-1, compare_op=GE, fill=0.0,
        )
        nc.gpsimd.affine_select(
            mtrail2[:], mtrail2[:], pattern=[[0, 2], [-1, 128]], base=0,
            channel_multiplier=1, compare_op=GE, fill=0.0,
        )
        MASKS = {"mlead": mlead2, "mtrail": mtrail2}

        bpat = msk_pool.tile([128, 1280 - XA], f32, tag="bpat", name="bpat")
        nc.gpsimd.memset(bpat[:], B_VALID)
        for r0, r1, mname in UNI_REGIONS:
            s0 = max(r0, XA)
            if s0 >= r1:
                continue
            w = r1 - s0
            shift = s0 - r0
            if mname == "mlead":  # keep (j_local + shift) - p >= 0
                nc.gpsimd.affine_select(
                    bpat[:, s0 - XA : r1 - XA], bpat[:, s0 - XA : r1 - XA],
                    pattern=[[1, w]], base=shift,
                    channel_multiplier=-1, compare_op=GE, fill=C_MASK,
                )
            else:  # keep p - (j_local + shift) >= 0
                nc.gpsimd.affine_select(
                    bpat[:, s0 - XA : r1 - XA], bpat[:, s0 - XA : r1 - XA],
                    pattern=[[-1, w]], base=-shift,
                    channel_multiplier=1, compare_op=GE, fill=C_MASK,
                )

        # late-needed inputs on the SWDGE queue after mask gen
        nc.gpsimd.dma_start(out=kt_sb[0][:, 768:N], in_=kt_d[0][:, 768:N])
        nc.gpsimd.dma_start(out=vx_sb[1][:], in_=vx_d[1])
        nc.gpsimd.dma_start(out=vx_sb[2][:], in_=vx_d[2])

        # ---- per-(head, phase) emitters ----
        pt_tiles = {}

        def emit_qk(u, h, pi):
            ring = rings[u % 2]
            for kc, off, w in phase_cols(pi, h):
                klhs = kh(h)[:, 128 * kc : 128 * kc + 128]
                qs = chunk_qs(kc)
                a = 0
                while a < w:  # split at ring bank boundaries
                    b = min(((off + a) // 512 + 1) * 512 - off, w)
                    nc.tensor.matmul(
                        ring[:, off + a : off + b],
                        klhs,
                        qh(h)[:, qs + a : qs + b],
                        start=True, stop=True, skip_group_check=True,
                    )
                    a = b

        def emit_exp(u, h, pi):
            ring = rings[u % 2]
            cols = phase_cols(pi, h)
            wtot = cols[-1][1] + cols[-1][2]
            if phase_uniform(pi):
                pa = pt_pool.tile([128, XA], bf16, tag="pt", name=f"pt_h{h}p{pi}")
                nc.scalar.activation(pa[:, :], ring[:, 0:XA], EXP)
                pd = pti_pool.tile(
                    [128, 1280 - XA], i32, tag="pti", name=f"pti_h{h}p{pi}"
                )
                nc.vector.scalar_tensor_tensor(
                    out=pd[:], in0=ring[:, XA:1280], scalar=A_CONST,
                    in1=bpat[:], op0=MULT, op1=ADD,
                )
                pt_tiles[(h, pi)] = (pa, pd)
            else:
                pa = pt_pool.tile([128, wtot], bf16, tag="pt", name=f"pt_h{h}p{pi}")
                nc.scalar.activation(pa[:, 0:wtot], ring[:, 0:wtot], EXP)
                pt_tiles[(h, pi)] = (pa, None)

        def ptslice(h, pi, kc, c0, c1):
            """bf16 AP for chunk-local cols [c0,c1) of chunk kc in phase pi."""
            pa, pd = pt_tiles[(h, pi)]
            off = dict((k, o) for k, o, _ in phase_cols(pi, h))[kc]
            a0, a1 = off + c0, off + c1
            if pd is None or a1 <= XA:
                return pa[:, a0:a1]
            assert a0 >= XA, f"piece straddles XA: {a0}..{a1}"
            return (
                pd[:, a0 - XA : a1 - XA]
                .bitcast(bf16)
                .rearrange("p (n two) -> p n two", two=2)[:, :, 1]
            )

        def mask_eng():
            return nc.gpsimd if MASK_ENG == "g" else nc.vector

        def emit_masks(h, pi):
            pa, pd = pt_tiles[(h, pi)]
            if phase_uniform(pi):
                for r0, r1, mname in UNI_REGIONS:
                    e = min(r1, XA)
                    if r0 >= e:
                        continue
                    m = MASKS[mname]
                    mask_eng().tensor_mul(
                        pa[:, r0:e], pa[:, r0:e], m[:, 0, 0 : e - r0]
                    )
                return
            for kc, off, w in phase_cols(pi, h):
                for c0, c1, mname in chunk_masks(kc):
                    m = MASKS[mname]
                    mask_eng().tensor_mul(
                        pa[:, off + c0 : off + c1],
                        pa[:, off + c0 : off + c1],
                        m[:, 0, 0 : c1 - c0],
                    )

        bank_tiles = {}
        otq = [nc.sync]

        def emit_pv(h, pi):
            for kc, a, b, full in PV_SCHED[pi]:
                cpi = kc // 2  # the chunk's own phase (deferred pieces: < pi)
                bank = a // 512
                key = (h, bank)
                if full:
                    assert key not in bank_tiles
                    bank_tiles[key] = po_pool.tile(
                        [65, 512], f32, tag="ot", name=f"ot_h{h}b{bank}"
                    )
                bt = bank_tiles[key]
                vstat = vx_sb[h][:, kc, :]
                qs = chunk_qs(kc)
                # split at the XA boundary inside uniform phases so each
                # matmul's pt slice is wholly ACT-side or DVE-side
                splits = [(a, b)]
                if phase_uniform(cpi):
                    off = dict((k, o) for k, o, _ in phase_cols(cpi, h))[kc]
                    qx = qs + (XA - off)  # q at the XA boundary
                    if a < qx < b:
                        splits = [(a, qx), (qx, b)]
                for s0, s1 in splits:
                    nc.tensor.matmul(
                        bt[:, s0 - 512 * bank : s1 - 512 * bank],
                        vstat,
                        ptslice(h, cpi, kc, s0 - qs, s1 - qs),
                        start=full and s0 == a, stop=False,
                        skip_group_check=True,
                    )
            done = [bank for bank, dpi in bank_done_for(h).items() if dpi == pi]
            if done:
                bank = done[0]
                bt = bank_tiles.pop((h, bank))
                osb = osb_pool.tile(
                    [65, 512], f32, tag="osb", name=f"osb_h{h}b{bank}"
                )
                if (h, bank) in COPY_ON_ACT:
                    nc.scalar.copy(out=osb[:], in_=bt[:])
                else:
                    nc.vector.tensor_copy(out=osb[:], in_=bt[:])
                q = otq[(4 * h + bank + h) % len(otq)]
                q.dma_start(
                    out=ot_d[h][:, 512 * bank : 512 * bank + 512], in_=osb[:]
                )

        # ---- software-pipelined emission: PV lags QK by PV_LAG units ----
        units = [(h, pi) for h in range(HPC) for pi in range(8)]
        L = len(units)
        for u, (h, pi) in enumerate(units):
            emit_qk(u, h, pi)
            emit_exp(u, h, pi)
            emit_masks(h, pi)
            if u >= PV_LAG:
                emit_pv(*units[u - PV_LAG])
            if u >= PV_LAG + 1:
                pt_tiles.pop(units[u - PV_LAG - 1])
        for u in range(L - PV_LAG, L):
            emit_pv(*units[u])
            if u >= 1:
                pt_tiles.pop(units[u - 1])
        pt_tiles.pop(units[L - 1])

    nc.compile()
    _CACHED_NC = nc
    return nc


# ---------------------------------------------------------------------------
# Entry points
# ---------------------------------------------------------------------------


def run(inputs, trace=False, trace_kwargs=None):
    """Returns (output [B,H,N,D] f32, BassKernelResults)."""
    from concourse import bass_utils

    Q = np.asarray(inputs["Q"], np.float32).reshape(B * H, N, D)
    K = np.asarray(inputs["K"], np.float32).reshape(B * H, N, D)
    V = np.asarray(inputs["V"], np.float32).reshape(B * H, N, D)
    in_maps = [prep_core_inputs(Q, K, V, c) for c in range(NCORES)]
    nc = build_module()
    res = bass_utils.run_bass_kernel_spmd(
        nc,
        in_maps,
        core_ids=list(range(NCORES)),
        trace=trace,
        **(trace_kwargs or {}),
    )
    ot_all = [res.results[c]["ot"] for c in range(NCORES)]
    og = host_global_rows(Q, K, V)
    gnum, gden = host_glob_strips(Q, K, V)
    return unprep_output(ot_all, og, gnum, gden), res


def kernel(**inputs) -> np.ndarray:
    out, _ = run(inputs, trace=False)
    return out


# revision 12
# speedup vs baseline: 1.0401x; 1.0095x over previous
"""Longformer attention Bass kernel for 8 TRN2 NeuronCores (v4).

Problem: B=2, H=16, N=2048, D=64, window=256, global positions 0..3.
Sharding: B*H = 32 heads -> 4 heads per core (head-parallel).

v4 changes over v3 (48.5us):
  - Zero-init PV matmuls removed: PV pieces are rescheduled so each O^T
    PSUM bank's FIRST writer is the full-bank piece of chunk 4k+1
    (start=True zeroes the whole 2KB bank); earlier chunks' pieces for
    that bank are deferred into that phase.  -3.4us of PE.
  - exp split between ACT and DVE: per uniform phase the ACT engine
    exps ring[:, 0:XA]; the DVE computes ring[:, XA:1280] with a fused
    Schraudolph bitcast-exp: one scalar_tensor_tensor
      i32 = int32(s * A + Bpat)   (store-convert does the 2^x trick)
    where Bpat is B_VALID on in-band entries and C_MASK on masked ones
    (masked entries land at ~2^-100: the band mask rides for free), and
    the bf16 P^T for the PV matmul is the upper 16 bits of each int32
    read through a stride-2 bitcast view (bf16 round via +2^15 in B).
    Schraudolph rel err ~1.8% RMS on the offloaded columns.
  - mask multiplies for the ACT part moved to GPSIMD (Pool), freeing
    the DVE; dead mc0 mask removed.
  - O^T bank copies split ACT/DVE by knob.
"""

import numpy as np
import ml_dtypes

B, H, N, D = 2, 16, 2048, 64
W = 256
NG = 4  # global positions 0..3
NCORES = 8
HPC = (B * H) // NCORES  # heads per core = 4
NKC = N // 128  # key chunks = 16
BF16 = ml_dtypes.bfloat16

# ---- tuning knobs ----------------------------------------------------------
XA = 640  # uniform-phase cols [0:XA) exp'd on ACT; [XA:1280) Schraudolph on DVE
SCHR_DELTA = 0.0575  # Schraudolph centering (fraction of 2^23)
A_CONST = float(np.float32((1 << 23) / np.log(2.0)))
B_VALID = float(np.float32((127 << 23) + 32768 - SCHR_DELTA * (1 << 23)))
C_MASK = float(np.float32(1 << 27))
# engine for bf16 mask multiplies on the ACT part: 'g' (Pool) or 'v' (DVE)
MASK_ENG = "g"
# (h, bank) pairs whose O^T copy runs on ACT instead of DVE: spread them so
# neither engine's in-order queue delays the ring-WAR critical chain
COPY_ON_ACT = {(0, 1), (1, 0), (1, 3), (2, 2), (3, 1), (3, 2)}
# PV emission lag in units: larger lag gives the QK->exp->mask->PV chain
# more pipeline slack before it gates the in-order PE program
PV_LAG = 3

# phase grouping of key chunks
PHASES = [[0, 1], [2, 3], [4, 5], [6, 7], [8, 9], [10, 11], [12, 13], [14, 15]]


def phases_for(h):
    return PHASES


# ot bank b is complete after the PV pass of this phase index
BANK_DONE_PHASE = {0: 2, 1: 4, 2: 6, 3: 7}


def bank_done_for(h):
    return BANK_DONE_PHASE


def chunk_qs(kc: int) -> int:
    if kc == 0:
        return 0
    return min(max(128 * kc - W, 0), N - 384)


def chunk_width(kc: int) -> int:
    if kc in (1, 14):
        return 512
    if kc in (0, 15):
        return 384
    return 640


def chunk_masks(kc: int):
    """Mask ops for chunk kc in chunk-local columns: (col0, col1, mask)."""
    if kc == 0:
        return [(256, 384, "mtrail")]
    if kc == 1:
        return [(384, 512, "mtrail")]
    if kc in (14, 15):
        return [(0, 128, "mlead")]
    return [(0, 128, "mlead"), (512, 640, "mtrail")]


def pv_pieces(kc: int):
    """PV output piece spans for chunk kc: (abs_col0, abs_col1)."""
    qs, w = chunk_qs(kc), chunk_width(kc)
    pieces = []
    a = qs
    while a < qs + w:
        b = min((a // 512 + 1) * 512, qs + w)
        pieces.append((a, b))
        a = b
    return pieces


def pv_schedule():
    """{phase: [(kc, a, b, is_full_bank_start)]} with each bank's first
    writer being the full-bank piece of chunk 4k+1 (start=True zeroes the
    bank); earlier chunks' pieces for that bank are deferred to that phase."""
    sched = {pi: [] for pi in range(8)}
    for kc in range(NKC):
        cp = kc // 2
        for a, b in pv_pieces(kc):
            bank = a // 512
            full = kc == 4 * bank + 1 and a == 512 * bank and b == a + 512
            p = cp if kc >= 4 * bank + 1 else 2 * bank
            sched[p].append((kc, a, b, full))
    for p in sched:
        sched[p].sort(key=lambda t: (not t[3], t[0], t[1]))
    return sched


PV_SCHED = pv_schedule()

# uniform-phase mask regions in absolute phase columns
UNI_REGIONS = [(0, 128, "mlead"), (512, 640, "mtrail"),
               (640, 768, "mlead"), (1152, 1280, "mtrail")]


def phase_cols(pi, h=0):
    """[(kc, col_off, width)] within the phase tile."""
    off = 0
    out = []
    for kc in phases_for(h)[pi]:
        w = chunk_width(kc)
        out.append((kc, off, w))
        off += w
    return out


def phase_uniform(pi):
    return all(chunk_width(kc) == 640 for kc in PHASES[pi])


# ---------------------------------------------------------------------------
# Numpy model of the exact device algorithm (geometry validation)
# ---------------------------------------------------------------------------


def _mask_tiles_np():
    p = np.arange(128)[:, None]
    j = np.arange(128)[None, :]
    return {
        "mlead": (j >= p).astype(np.float32),
        "mtrail": (j <= p).astype(np.float32),
    }


def _bpat_np():
    """[128, 1280-XA] f32 Schraudolph bias pattern for uniform phases."""
    bp = np.full((128, 1280 - XA), np.float32(B_VALID), np.float32)
    masks = _mask_tiles_np()
    for r0, r1, mname in UNI_REGIONS:
        s0, s1 = max(r0, XA), r1
        if s0 >= s1:
            continue
        p = np.arange(128)[:, None]
        j = np.arange(s0 - r0, s1 - r0)[None, :]
        keep = (j >= p) if mname == "mlead" else (j <= p)
        bp[:, s0 - XA : s1 - XA] = np.where(keep, np.float32(B_VALID),
                                            np.float32(C_MASK))
    return bp


def _schraudolph_np(st, bpat):
    y = st.astype(np.float32) * np.float32(A_CONST) + bpat
    i = y.astype(np.int32)
    return ((i.view(np.uint32) >> np.uint32(16)).astype(np.uint16)
            .view(BF16).astype(np.float32))


def numpy_model_head(qT, kT, vx):
    """qT/kT: [64, N] bf16-rounded f32 (q pre-scaled); vx: [N, 65] bf16-rounded.

    Returns OT [65, N] f32 (unnormalized band-only O^T + denominator row).
    """
    qT = qT.astype(np.float32)
    kT = kT.astype(np.float32)
    vx = vx.astype(np.float32)
    masks = _mask_tiles_np()
    bpat = _bpat_np()
    ot = np.zeros((65, N), np.float32)
    for pi in range(8):
        cols = phase_cols(pi)
        wtot = cols[-1][1] + cols[-1][2]
        st = np.empty((128, wtot), np.float32)
        for kc, off, w in cols:
            qs = chunk_qs(kc)
            kk = slice(128 * kc, 128 * kc + 128)
            st[:, off : off + w] = kT[:, kk].T @ qT[:, qs : qs + w]
        if phase_uniform(pi):
            pt = np.empty((128, wtot), np.float32)
            pt[:, :XA] = np.exp(st[:, :XA]).astype(BF16).astype(np.float32)
            pt[:, XA:] = _schraudolph_np(st[:, XA:], bpat)
            for r0, r1, mname in UNI_REGIONS:
                e = min(r1, XA)
                if r0 < e:
                    pt[:, r0:e] *= masks[mname][:, : e - r0]
        else:
            pt = np.exp(st).astype(BF16).astype(np.float32)
            for kc, off, w in cols:
                for c0, c1, mname in chunk_masks(kc):
                    pt[:, off + c0 : off + c1] *= masks[mname][:, : c1 - c0]
        pt = pt.astype(BF16).astype(np.float32)
        for kc, off, w in cols:
            qs = chunk_qs(kc)
            kk = slice(128 * kc, 128 * kc + 128)
            ot[:, qs : qs + w] += vx[kk].T @ pt[:, off : off + w]
    return ot


# ---------------------------------------------------------------------------
# Host-side prep / unprep
# ---------------------------------------------------------------------------


def prep_core_inputs(Q, K, V, core):
    """Q/K/V: [B*H, N, D] f32. Returns the in_map for one core."""
    h0 = core * HPC
    qt = np.empty((2, 128, N), BF16)
    kt = np.empty((2, 128, N), BF16)
    vx = np.zeros((HPC, 128, NKC, 65), BF16)
    for p in range(2):
        for s in range(2):
            h = h0 + 2 * p + s
            qt[p, 64 * s : 64 * s + 64] = (Q[h].T * np.float32(0.125)).astype(BF16)
            kt[p, 64 * s : 64 * s + 64] = K[h].T.astype(BF16)
    for i in range(HPC):
        v = np.concatenate([V[h0 + i], np.ones((N, 1), np.float32)], axis=1)
        vx[i] = v.reshape(NKC, 128, 65).transpose(1, 0, 2).astype(BF16)
    return {"qt": qt, "kt": kt, "vx": vx}


def host_glob_strips(Q, K, V):
    """f32 contributions of the 4 global KEYS beyond the window (k < q-256)."""
    scale = np.float32(0.125)
    s = np.einsum("hqd,hkd->hqk", Q[:, 256:].astype(np.float32), K[:, 0:NG]) * scale
    e = np.exp(s)  # [BH, N-256, NG]
    q_abs = np.arange(256, N)[None, :, None]
    k_idx = np.arange(NG)[None, None, :]
    e = e * (k_idx < q_abs - 256)
    gnum = np.einsum("hqk,hkd->hqd", e, V[:, 0:NG])
    gden = e.sum(axis=-1)
    return gnum, gden


def host_global_rows(Q, K, V):
    """Exact f32 attention for the 4 global query rows of every head."""
    scale = np.float32(1.0 / np.sqrt(D))
    s = np.einsum("hqd,hkd->hqk", Q[:, :NG].astype(np.float32), K) * scale
    s -= s.max(axis=-1, keepdims=True)
    p = np.exp(s)
    p /= p.sum(axis=-1, keepdims=True)
    return np.einsum("hqk,hkd->hqd", p, V)


def unprep_output(ot_all, og, gnum, gden):
    out = np.empty((B * H, N, D), np.float32)
    for core in range(NCORES):
        ot = np.asarray(ot_all[core])
        for i in range(HPC):
            h = core * HPC + i
            num = ot[i, :D, :].T.copy()  # [N, D]
            den = ot[i, D, :].copy()  # [N]
            num[256:] += gnum[h]
            den[256:] += gden[h]
            out[h] = num / den[:, None]
    out[:, 0:NG] = og
    return out.reshape(B, H, N, D)


# ---------------------------------------------------------------------------
# Bass module
# ---------------------------------------------------------------------------

_CACHED_NC = None


def build_module():
    global _CACHED_NC
    if _CACHED_NC is not None:
        return _CACHED_NC
    from contextlib import ExitStack

    import concourse.bass as bass  # noqa: F401
    import concourse.tile as tile
    from concourse import bacc, mybir

    f32 = mybir.dt.float32
    bf16 = mybir.dt.bfloat16
    i32 = mybir.dt.int32
    EXP = mybir.ActivationFunctionType.Exp
    GE = mybir.AluOpType.is_ge
    MULT = mybir.AluOpType.mult
    ADD = mybir.AluOpType.add

    nc = bacc.Bacc("TRN2", target_bir_lowering=False, debug=False)
    qt_d = nc.dram_tensor("qt", [2, 128, N], bf16, kind="ExternalInput")
    kt_d = nc.dram_tensor("kt", [2, 128, N], bf16, kind="ExternalInput")
    vx_d = nc.dram_tensor("vx", [HPC, 128, NKC, 65], bf16, kind="ExternalInput")
    ot_d = nc.dram_tensor("ot", [HPC, 65, N], f32, kind="ExternalOutput")

    with tile.TileContext(nc) as tc, ExitStack() as ctx:
        io_pool = ctx.enter_context(tc.tile_pool(name="io", bufs=1))
        msk_pool = ctx.enter_context(tc.tile_pool(name="msk", bufs=1))
        pt_pool = ctx.enter_context(tc.tile_pool(name="ptp", bufs=PV_LAG + 2))
        pti_pool = ctx.enter_context(tc.tile_pool(name="pti", bufs=PV_LAG + 2))
        osb_pool = ctx.enter_context(tc.tile_pool(name="osb", bufs=3))
        ring_pool = ctx.enter_context(tc.tile_pool(name="ring", bufs=1, space="PSUM"))
        po_pool = ctx.enter_context(tc.tile_pool(name="po", bufs=2, space="PSUM"))

        # ---- static PSUM: two 3-bank score rings ----
        ringA = ring_pool.tile([128, 1536], f32, tag="ringA", name="ringA")
        ringB = ring_pool.tile([128, 1536], f32, tag="ringB", name="ringB")
        rings = [ringA, ringB]

        # ---- inputs ----
        qt_sb = []
        kt_sb = []
        vx_sb = []
        for pair in range(2):
            qtp = io_pool.tile([128, N], bf16, tag=f"qt{pair}", name=f"qt{pair}")
            ktp = io_pool.tile([128, N], bf16, tag=f"kt{pair}", name=f"kt{pair}")
            qt_sb.append(qtp)
            kt_sb.append(ktp)
        for h in range(HPC):
            vxh = io_pool.tile([128, NKC, 65], bf16, tag=f"vx{h}", name=f"vx{h}")
            vx_sb.append(vxh)
        # issue order == transfer order; tiny pair-0 lead pieces unblock the
        # first QK phase as early as possible, everything else streams
        nc.sync.dma_start(out=kt_sb[0][:, 0:256], in_=kt_d[0][:, 0:256])
        nc.scalar.dma_start(out=qt_sb[0][:, 0:512], in_=qt_d[0][:, 0:512])
        nc.sync.dma_start(out=kt_sb[0][:, 256:768], in_=kt_d[0][:, 256:768])
        nc.scalar.dma_start(out=qt_sb[0][:, 512:1152], in_=qt_d[0][:, 512:1152])
        nc.sync.dma_start(out=qt_sb[0][:, 1152:N], in_=qt_d[0][:, 1152:N])
        nc.scalar.dma_start(out=vx_sb[0][:], in_=vx_d[0])
        nc.sync.dma_start(out=kt_sb[1][:], in_=kt_d[1])
        nc.scalar.dma_start(out=qt_sb[1][:], in_=qt_d[1])
        nc.sync.dma_start(out=vx_sb[3][:], in_=vx_d[3])

        # ---- warm the PE pstate while the first DMAs land ----
        wu = msk_pool.tile([64, 512], bf16, tag="wu", name="wu")
        nc.vector.memset(wu[:, 0:128], 0.0)
        nc.tensor.matmul(
            ringB[:, 0:128], wu[:, 0:128], wu[:, 0:128],
            start=True, stop=True, skip_group_check=True,
        )
        nc.vector.memset(wu[:, 128:512], 0.0)
        for i in range(5):
            nc.tensor.matmul(
                ringB[:, 0:512], wu[:, 0:128], wu[:],
                start=True, stop=True, skip_group_check=True,
            )

        def qh(h):
            return qt_sb[h // 2][64 * (h % 2) : 64 * (h % 2) + 64, :]

        def kh(h):
            return kt_sb[h // 2][64 * (h % 2) : 64 * (h % 2) + 64, :]

        # ---- mask tiles (0/1 bf16) + Schraudolph bias pattern (f32) ----
        mlead2 = msk_pool.tile([128, 2, 128], bf16, tag="mlead2", name="mlead2")
        mtrail2 = msk_pool.tile([128, 2, 128], bf16, tag="mtrail2", name="mtrail2")
        nc.gpsimd.memset(mlead2[:], 1.0)
        nc.gpsimd.memset(mtrail2[:], 1.0)
        nc.gpsimd.affine_select(
            mlead2[:], mlead2[:], pattern=[[0, 2], [1, 128]], base=0,
            channel_multiplier=-1, compare_op=GE, fill=0.0,
        )
        nc.gpsimd.affine_select(
            mtrail2[:], mtrail2[:], pattern=[[0, 2], [-1, 128]], base=0,
            channel_multiplier=1, compare_op=GE, fill=0.0,
        )
        MASKS = {"mlead": mlead2, "mtrail": mtrail2}

        bpat = msk_pool.tile([128, 1280 - XA], f32, tag="bpat", name="bpat")
        nc.gpsimd.memset(bpat[:], B_VALID)
        for r0, r1, mname in UNI_REGIONS:
            s0 = max(r0, XA)
            if s0 >= r1:
                continue
            w = r1 - s0
            shift = s0 - r0
            if mname == "mlead":  # keep (j_local + shift) - p >= 0
                nc.gpsimd.affine_select(
                    bpat[:, s0 - XA : r1 - XA], bpat[:, s0 - XA : r1 - XA],
                    pattern=[[1, w]], base=shift,
                    channel_multiplier=-1, compare_op=GE, fill=C_MASK,
                )
            else:  # keep p - (j_local + shift) >= 0
                nc.gpsimd.affine_select(
                    bpat[:, s0 - XA : r1 - XA], bpat[:, s0 - XA : r1 - XA],
                    pattern=[[-1, w]], base=-shift,
                    channel_multiplier=1, compare_op=GE, fill=C_MASK,
                )

        # late-needed inputs on the SWDGE queue after mask gen
        nc.gpsimd.dma_start(out=kt_sb[0][:, 768:N], in_=kt_d[0][:, 768:N])
        nc.gpsimd.dma_start(out=vx_sb[1][:], in_=vx_d[1])
        nc.gpsimd.dma_start(out=vx_sb[2][:], in_=vx_d[2])

        # ---- per-(head, phase) emitters ----
        pt_tiles = {}

        def emit_qk(u, h, pi):
            ring = rings[u % 2]
            for kc, off, w in phase_cols(pi, h):
                klhs = kh(h)[:, 128 * kc : 128 * kc + 128]
                qs = chunk_qs(kc)
                a = 0
                while a < w:  # split at ring bank boundaries
                    b = min(((off + a) // 512 + 1) * 512 - off, w)
                    nc.tensor.matmul(
                        ring[:, off + a : off + b],
                        klhs,
                        qh(h)[:, qs + a : qs + b],
                        start=True, stop=True, skip_group_check=True,
                    )
                    a = b

        def emit_exp(u, h, pi):
            ring = rings[u % 2]
            cols = phase_cols(pi, h)
            wtot = cols[-1][1] + cols[-1][2]
            if phase_uniform(pi):
                pa = pt_pool.tile([128, XA], bf16, tag="pt", name=f"pt_h{h}p{pi}")
                nc.scalar.activation(pa[:, :], ring[:, 0:XA], EXP)
                pd = pti_pool.tile(
                    [128, 1280 - XA], i32, tag="pti", name=f"pti_h{h}p{pi}"
                )
                nc.vector.scalar_tensor_tensor(
                    out=pd[:], in0=ring[:, XA:1280], scalar=A_CONST,
                    in1=bpat[:], op0=MULT, op1=ADD,
                )
                pt_tiles[(h, pi)] = (pa, pd)
            else:
                pa = pt_pool.tile([128, wtot], bf16, tag="pt", name=f"pt_h{h}p{pi}")
                nc.scalar.activation(pa[:, 0:wtot], ring[:, 0:wtot], EXP)
                pt_tiles[(h, pi)] = (pa, None)

        def ptslice(h, pi, kc, c0, c1):
            """bf16 AP for chunk-local cols [c0,c1) of chunk kc in phase pi."""
            pa, pd = pt_tiles[(h, pi)]
            off = dict((k, o) for k, o, _ in phase_cols(pi, h))[kc]
            a0, a1 = off + c0, off + c1
            if pd is None or a1 <= XA:
                return pa[:, a0:a1]
            assert a0 >= XA, f"piece straddles XA: {a0}..{a1}"
            return (
                pd[:, a0 - XA : a1 - XA]
                .bitcast(bf16)
                .rearrange("p (n two) -> p n two", two=2)[:, :, 1]
            )

        def mask_eng():
            return nc.gpsimd if MASK_ENG == "g" else nc.vector

        def emit_masks(h, pi):
            pa, pd = pt_tiles[(h, pi)]
            if phase_uniform(pi):
                for r0, r1, mname in UNI_REGIONS:
                    e = min(r1, XA)
                    if r0 >= e:
                        continue
                    m = MASKS[mname]
                    mask_eng().tensor_mul(
                        pa[:, r0:e], pa[:, r0:e], m[:, 0, 0 : e - r0]
                    )
                return
            for kc, off, w in phase_cols(pi, h):
                for c0, c1, mname in chunk_masks(kc):
                    m = MASKS[mname]
                    mask_eng().tensor_mul(
                        pa[:, off + c0 : off + c1],
                        pa[:, off + c0 : off + c1],
                        m[:, 0, 0 : c1 - c0],
                    )

        bank_tiles = {}
        otq = [nc.sync]

        def emit_pv(h, pi):
            for kc, a, b, full in PV_SCHED[pi]:
                cpi = kc // 2  # the chunk's own phase (deferred pieces: < pi)
                bank = a // 512
                key = (h, bank)
                if full:
                    assert key not in bank_tiles
                    bank_tiles[key] = po_pool.tile(
                        [65, 512], f32, tag="ot", name=f"ot_h{h}b{bank}"
                    )
                bt = bank_tiles[key]
                vstat = vx_sb[h][:, kc, :]
                qs = chunk_qs(kc)
                # split at the XA boundary inside uniform phases so each
                # matmul's pt slice is wholly ACT-side or DVE-side
                splits = [(a, b)]
                if phase_uniform(cpi):
                    off = dict((k, o) for k, o, _ in phase_cols(cpi, h))[kc]
                    qx = qs + (XA - off)  # q at the XA boundary
                    if a < qx < b:
                        splits = [(a, qx), (qx, b)]
                for s0, s1 in splits:
                    nc.tensor.matmul(
                        bt[:, s0 - 512 * bank : s1 - 512 * bank],
                        vstat,
                        ptslice(h, cpi, kc, s0 - qs, s1 - qs),
                        start=full and s0 == a, stop=False,
                        skip_group_check=True,
                    )
            done = [bank for bank, dpi in bank_done_for(h).items() if dpi == pi]
            if done:
                bank = done[0]
                bt = bank_tiles.pop((h, bank))
                osb = osb_pool.tile(
                    [65, 512], f32, tag="osb", name=f"osb_h{h}b{bank}"
                )
                # split the copy across ACT and DVE so neither engine's
                # in-order queue inserts a long op ahead of the ring-critical
                # activation / Schraudolph instructions
                nc.scalar.copy(out=osb[:, 0:256], in_=bt[:, 0:256])
                nc.vector.tensor_copy(out=osb[:, 256:512], in_=bt[:, 256:512])
                q = otq[(4 * h + bank + h) % len(otq)]
                q.dma_start(
                    out=ot_d[h][:, 512 * bank : 512 * bank + 512], in_=osb[:]
                )

        # ---- software-pipelined emission: PV lags QK by PV_LAG units ----
        units = [(h, pi) for h in range(HPC) for pi in range(8)]
        L = len(units)
        for u, (h, pi) in enumerate(units):
            emit_qk(u, h, pi)
            emit_exp(u, h, pi)
            emit_masks(h, pi)
            if u >= PV_LAG:
                emit_pv(*units[u - PV_LAG])
            if u >= PV_LAG + 1:
                pt_tiles.pop(units[u - PV_LAG - 1])
        for u in range(L - PV_LAG, L):
            emit_pv(*units[u])
            if u >= 1:
                pt_tiles.pop(units[u - 1])
        pt_tiles.pop(units[L - 1])

    nc.compile()
    _CACHED_NC = nc
    return nc


# ---------------------------------------------------------------------------
# Entry points
# ---------------------------------------------------------------------------


def run(inputs, trace=False, trace_kwargs=None):
    """Returns (output [B,H,N,D] f32, BassKernelResults)."""
    from concourse import bass_utils

    Q = np.asarray(inputs["Q"], np.float32).reshape(B * H, N, D)
    K = np.asarray(inputs["K"], np.float32).reshape(B * H, N, D)
    V = np.asarray(inputs["V"], np.float32).reshape(B * H, N, D)
    in_maps = [prep_core_inputs(Q, K, V, c) for c in range(NCORES)]
    nc = build_module()
    res = bass_utils.run_bass_kernel_spmd(
        nc,
        in_maps,
        core_ids=list(range(NCORES)),
        trace=trace,
        **(trace_kwargs or {}),
    )
    ot_all = [res.results[c]["ot"] for c in range(NCORES)]
    og = host_global_rows(Q, K, V)
    gnum, gden = host_glob_strips(Q, K, V)
    return unprep_output(ot_all, og, gnum, gden), res


def kernel(**inputs) -> np.ndarray:
    out, _ = run(inputs, trace=False)
    return out


# revision 13
# speedup vs baseline: 1.0412x; 1.0011x over previous
"""Longformer attention Bass kernel for 8 TRN2 NeuronCores (v4).

Problem: B=2, H=16, N=2048, D=64, window=256, global positions 0..3.
Sharding: B*H = 32 heads -> 4 heads per core (head-parallel).

v4 changes over v3 (48.5us):
  - Zero-init PV matmuls removed: PV pieces are rescheduled so each O^T
    PSUM bank's FIRST writer is the full-bank piece of chunk 4k+1
    (start=True zeroes the whole 2KB bank); earlier chunks' pieces for
    that bank are deferred into that phase.  -3.4us of PE.
  - exp split between ACT and DVE: per uniform phase the ACT engine
    exps ring[:, 0:XA]; the DVE computes ring[:, XA:1280] with a fused
    Schraudolph bitcast-exp: one scalar_tensor_tensor
      i32 = int32(s * A + Bpat)   (store-convert does the 2^x trick)
    where Bpat is B_VALID on in-band entries and C_MASK on masked ones
    (masked entries land at ~2^-100: the band mask rides for free), and
    the bf16 P^T for the PV matmul is the upper 16 bits of each int32
    read through a stride-2 bitcast view (bf16 round via +2^15 in B).
    Schraudolph rel err ~1.8% RMS on the offloaded columns.
  - mask multiplies for the ACT part moved to GPSIMD (Pool), freeing
    the DVE; dead mc0 mask removed.
  - O^T bank copies split ACT/DVE by knob.
"""

import numpy as np
import ml_dtypes

B, H, N, D = 2, 16, 2048, 64
W = 256
NG = 4  # global positions 0..3
NCORES = 8
HPC = (B * H) // NCORES  # heads per core = 4
NKC = N // 128  # key chunks = 16
BF16 = ml_dtypes.bfloat16

# ---- tuning knobs ----------------------------------------------------------
XA = 640  # uniform-phase cols [0:XA) exp'd on ACT; [XA:1280) Schraudolph on DVE
SCHR_DELTA = 0.0575  # Schraudolph centering (fraction of 2^23)
A_CONST = float(np.float32((1 << 23) / np.log(2.0)))
B_VALID = float(np.float32((127 << 23) + 32768 - SCHR_DELTA * (1 << 23)))
C_MASK = float(np.float32(1 << 27))
# engine for bf16 mask multiplies on the ACT part: 'g' (Pool) or 'v' (DVE)
MASK_ENG = "g"
# (h, bank) pairs whose O^T copy runs on ACT instead of DVE: spread them so
# neither engine's in-order queue delays the ring-WAR critical chain
COPY_ON_ACT = {(0, 1), (1, 0), (1, 3), (2, 2), (3, 1), (3, 2)}
# PV emission lag in units: larger lag gives the QK->exp->mask->PV chain
# more pipeline slack before it gates the in-order PE program
PV_LAG = 3

# phase grouping of key chunks
PHASES = [[0, 1], [2, 3], [4, 5], [6, 7], [8, 9], [10, 11], [12, 13], [14, 15]]


def phases_for(h):
    return PHASES


# ot bank b is complete after the PV pass of this phase index
BANK_DONE_PHASE = {0: 2, 1: 4, 2: 6, 3: 7}


def bank_done_for(h):
    return BANK_DONE_PHASE


def chunk_qs(kc: int) -> int:
    if kc == 0:
        return 0
    return min(max(128 * kc - W, 0), N - 384)


def chunk_width(kc: int) -> int:
    if kc in (1, 14):
        return 512
    if kc in (0, 15):
        return 384
    return 640


def chunk_masks(kc: int):
    """Mask ops for chunk kc in chunk-local columns: (col0, col1, mask)."""
    if kc == 0:
        return [(256, 384, "mtrail")]
    if kc == 1:
        return [(384, 512, "mtrail")]
    if kc in (14, 15):
        return [(0, 128, "mlead")]
    return [(0, 128, "mlead"), (512, 640, "mtrail")]


def pv_pieces(kc: int):
    """PV output piece spans for chunk kc: (abs_col0, abs_col1)."""
    qs, w = chunk_qs(kc), chunk_width(kc)
    pieces = []
    a = qs
    while a < qs + w:
        b = min((a // 512 + 1) * 512, qs + w)
        pieces.append((a, b))
        a = b
    return pieces


def pv_schedule():
    """{phase: [(kc, a, b, is_full_bank_start)]} with each bank's first
    writer being the full-bank piece of chunk 4k+1 (start=True zeroes the
    bank); earlier chunks' pieces for that bank are deferred to that phase."""
    sched = {pi: [] for pi in range(8)}
    for kc in range(NKC):
        cp = kc // 2
        for a, b in pv_pieces(kc):
            bank = a // 512
            full = kc == 4 * bank + 1 and a == 512 * bank and b == a + 512
            p = cp if kc >= 4 * bank + 1 else 2 * bank
            sched[p].append((kc, a, b, full))
    for p in sched:
        sched[p].sort(key=lambda t: (not t[3], t[0], t[1]))
    return sched


PV_SCHED = pv_schedule()

# uniform-phase mask regions in absolute phase columns
UNI_REGIONS = [(0, 128, "mlead"), (512, 640, "mtrail"),
               (640, 768, "mlead"), (1152, 1280, "mtrail")]


def phase_cols(pi, h=0):
    """[(kc, col_off, width)] within the phase tile."""
    off = 0
    out = []
    for kc in phases_for(h)[pi]:
        w = chunk_width(kc)
        out.append((kc, off, w))
        off += w
    return out


def phase_uniform(pi):
    return all(chunk_width(kc) == 640 for kc in PHASES[pi])


# ---------------------------------------------------------------------------
# Numpy model of the exact device algorithm (geometry validation)
# ---------------------------------------------------------------------------


def _mask_tiles_np():
    p = np.arange(128)[:, None]
    j = np.arange(128)[None, :]
    return {
        "mlead": (j >= p).astype(np.float32),
        "mtrail": (j <= p).astype(np.float32),
    }


def _bpat_np():
    """[128, 1280-XA] f32 Schraudolph bias pattern for uniform phases."""
    bp = np.full((128, 1280 - XA), np.float32(B_VALID), np.float32)
    masks = _mask_tiles_np()
    for r0, r1, mname in UNI_REGIONS:
        s0, s1 = max(r0, XA), r1
        if s0 >= s1:
            continue
        p = np.arange(128)[:, None]
        j = np.arange(s0 - r0, s1 - r0)[None, :]
        keep = (j >= p) if mname == "mlead" else (j <= p)
        bp[:, s0 - XA : s1 - XA] = np.where(keep, np.float32(B_VALID),
                                            np.float32(C_MASK))
    return bp


def _schraudolph_np(st, bpat):
    y = st.astype(np.float32) * np.float32(A_CONST) + bpat
    i = y.astype(np.int32)
    return ((i.view(np.uint32) >> np.uint32(16)).astype(np.uint16)
            .view(BF16).astype(np.float32))


def numpy_model_head(qT, kT, vx):
    """qT/kT: [64, N] bf16-rounded f32 (q pre-scaled); vx: [N, 65] bf16-rounded.

    Returns OT [65, N] f32 (unnormalized band-only O^T + denominator row).
    """
    qT = qT.astype(np.float32)
    kT = kT.astype(np.float32)
    vx = vx.astype(np.float32)
    masks = _mask_tiles_np()
    bpat = _bpat_np()
    ot = np.zeros((65, N), np.float32)
    for pi in range(8):
        cols = phase_cols(pi)
        wtot = cols[-1][1] + cols[-1][2]
        st = np.empty((128, wtot), np.float32)
        for kc, off, w in cols:
            qs = chunk_qs(kc)
            kk = slice(128 * kc, 128 * kc + 128)
            st[:, off : off + w] = kT[:, kk].T @ qT[:, qs : qs + w]
        if phase_uniform(pi):
            pt = np.empty((128, wtot), np.float32)
            pt[:, :XA] = np.exp(st[:, :XA]).astype(BF16).astype(np.float32)
            pt[:, XA:] = _schraudolph_np(st[:, XA:], bpat)
            for r0, r1, mname in UNI_REGIONS:
                e = min(r1, XA)
                if r0 < e:
                    pt[:, r0:e] *= masks[mname][:, : e - r0]
        else:
            pt = np.exp(st).astype(BF16).astype(np.float32)
            for kc, off, w in cols:
                for c0, c1, mname in chunk_masks(kc):
                    pt[:, off + c0 : off + c1] *= masks[mname][:, : c1 - c0]
        pt = pt.astype(BF16).astype(np.float32)
        for kc, off, w in cols:
            qs = chunk_qs(kc)
            kk = slice(128 * kc, 128 * kc + 128)
            ot[:, qs : qs + w] += vx[kk].T @ pt[:, off : off + w]
    return ot


# ---------------------------------------------------------------------------
# Host-side prep / unprep
# ---------------------------------------------------------------------------


def prep_core_inputs(Q, K, V, core):
    """Q/K/V: [B*H, N, D] f32. Returns the in_map for one core."""
    h0 = core * HPC
    qt = np.empty((2, 128, N), BF16)
    kt = np.empty((2, 128, N), BF16)
    vx = np.zeros((HPC, 128, NKC, 65), BF16)
    for p in range(2):
        for s in range(2):
            h = h0 + 2 * p + s
            qt[p, 64 * s : 64 * s + 64] = (Q[h].T * np.float32(0.125)).astype(BF16)
            kt[p, 64 * s : 64 * s + 64] = K[h].T.astype(BF16)
    for i in range(HPC):
        v = np.concatenate([V[h0 + i], np.ones((N, 1), np.float32)], axis=1)
        vx[i] = v.reshape(NKC, 128, 65).transpose(1, 0, 2).astype(BF16)
    return {"qt": qt, "kt": kt, "vx": vx}


def host_glob_strips(Q, K, V):
    """f32 contributions of the 4 global KEYS beyond the window (k < q-256)."""
    scale = np.float32(0.125)
    s = np.einsum("hqd,hkd->hqk", Q[:, 256:].astype(np.float32), K[:, 0:NG]) * scale
    e = np.exp(s)  # [BH, N-256, NG]
    q_abs = np.arange(256, N)[None, :, None]
    k_idx = np.arange(NG)[None, None, :]
    e = e * (k_idx < q_abs - 256)
    gnum = np.einsum("hqk,hkd->hqd", e, V[:, 0:NG])
    gden = e.sum(axis=-1)
    return gnum, gden


def host_global_rows(Q, K, V):
    """Exact f32 attention for the 4 global query rows of every head."""
    scale = np.float32(1.0 / np.sqrt(D))
    s = np.einsum("hqd,hkd->hqk", Q[:, :NG].astype(np.float32), K) * scale
    s -= s.max(axis=-1, keepdims=True)
    p = np.exp(s)
    p /= p.sum(axis=-1, keepdims=True)
    return np.einsum("hqk,hkd->hqd", p, V)


def unprep_output(ot_all, og, gnum, gden):
    out = np.empty((B * H, N, D), np.float32)
    for core in range(NCORES):
        ot = np.asarray(ot_all[core])
        for i in range(HPC):
            h = core * HPC + i
            num = ot[i, :D, :].T.copy()  # [N, D]
            den = ot[i, D, :].copy()  # [N]
            num[256:] += gnum[h]
            den[256:] += gden[h]
            out[h] = num / den[:, None]
    out[:, 0:NG] = og
    return out.reshape(B, H, N, D)


# ---------------------------------------------------------------------------
# Bass module
# ---------------------------------------------------------------------------

_CACHED_NC = None


def build_module():
    global _CACHED_NC
    if _CACHED_NC is not None:
        return _CACHED_NC
    from contextlib import ExitStack

    import concourse.bass as bass  # noqa: F401
    import concourse.tile as tile
    from concourse import bacc, mybir

    f32 = mybir.dt.float32
    bf16 = mybir.dt.bfloat16
    i32 = mybir.dt.int32
    EXP = mybir.ActivationFunctionType.Exp
    GE = mybir.AluOpType.is_ge
    MULT = mybir.AluOpType.mult
    ADD = mybir.AluOpType.add

    nc = bacc.Bacc("TRN2", target_bir_lowering=False, debug=False)
    qt_d = nc.dram_tensor("qt", [2, 128, N], bf16, kind="ExternalInput")
    kt_d = nc.dram_tensor("kt", [2, 128, N], bf16, kind="ExternalInput")
    vx_d = nc.dram_tensor("vx", [HPC, 128, NKC, 65], bf16, kind="ExternalInput")
    ot_d = nc.dram_tensor("ot", [HPC, 65, N], f32, kind="ExternalOutput")

    with tile.TileContext(nc) as tc, ExitStack() as ctx:
        io_pool = ctx.enter_context(tc.tile_pool(name="io", bufs=1))
        msk_pool = ctx.enter_context(tc.tile_pool(name="msk", bufs=1))
        pt_pool = ctx.enter_context(tc.tile_pool(name="ptp", bufs=PV_LAG + 4))
        pti_pool = ctx.enter_context(tc.tile_pool(name="pti", bufs=PV_LAG + 4))
        osb_pool = ctx.enter_context(tc.tile_pool(name="osb", bufs=3))
        ring_pool = ctx.enter_context(tc.tile_pool(name="ring", bufs=1, space="PSUM"))
        po_pool = ctx.enter_context(tc.tile_pool(name="po", bufs=2, space="PSUM"))

        # ---- static PSUM: two 3-bank score rings ----
        ringA = ring_pool.tile([128, 1536], f32, tag="ringA", name="ringA")
        ringB = ring_pool.tile([128, 1536], f32, tag="ringB", name="ringB")
        rings = [ringA, ringB]

        # ---- inputs ----
        qt_sb = []
        kt_sb = []
        vx_sb = []
        for pair in range(2):
            qtp = io_pool.tile([128, N], bf16, tag=f"qt{pair}", name=f"qt{pair}")
            ktp = io_pool.tile([128, N], bf16, tag=f"kt{pair}", name=f"kt{pair}")
            qt_sb.append(qtp)
            kt_sb.append(ktp)
        for h in range(HPC):
            vxh = io_pool.tile([128, NKC, 65], bf16, tag=f"vx{h}", name=f"vx{h}")
            vx_sb.append(vxh)
        # issue order == transfer order; tiny pair-0 lead pieces unblock the
        # first QK phase as early as possible, everything else streams
        nc.sync.dma_start(out=kt_sb[0][:, 0:256], in_=kt_d[0][:, 0:256])
        nc.scalar.dma_start(out=qt_sb[0][:, 0:512], in_=qt_d[0][:, 0:512])
        nc.sync.dma_start(out=kt_sb[0][:, 256:768], in_=kt_d[0][:, 256:768])
        nc.scalar.dma_start(out=qt_sb[0][:, 512:1152], in_=qt_d[0][:, 512:1152])
        nc.sync.dma_start(out=qt_sb[0][:, 1152:N], in_=qt_d[0][:, 1152:N])
        nc.scalar.dma_start(out=vx_sb[0][:], in_=vx_d[0])
        nc.sync.dma_start(out=kt_sb[1][:], in_=kt_d[1])
        nc.scalar.dma_start(out=qt_sb[1][:], in_=qt_d[1])
        nc.sync.dma_start(out=vx_sb[3][:], in_=vx_d[3])

        # ---- warm the PE pstate while the first DMAs land ----
        wu = msk_pool.tile([64, 512], bf16, tag="wu", name="wu")
        nc.vector.memset(wu[:, 0:128], 0.0)
        nc.tensor.matmul(
            ringB[:, 0:128], wu[:, 0:128], wu[:, 0:128],
            start=True, stop=True, skip_group_check=True,
        )
        nc.vector.memset(wu[:, 128:512], 0.0)
        for i in range(5):
            nc.tensor.matmul(
                ringB[:, 0:512], wu[:, 0:128], wu[:],
                start=True, stop=True, skip_group_check=True,
            )

        def qh(h):
            return qt_sb[h // 2][64 * (h % 2) : 64 * (h % 2) + 64, :]

        def kh(h):
            return kt_sb[h // 2][64 * (h % 2) : 64 * (h % 2) + 64, :]

        # ---- mask tiles (0/1 bf16) + Schraudolph bias pattern (f32) ----
        mlead2 = msk_pool.tile([128, 2, 128], bf16, tag="mlead2", name="mlead2")
        mtrail2 = msk_pool.tile([128, 2, 128], bf16, tag="mtrail2", name="mtrail2")
        nc.gpsimd.memset(mlead2[:], 1.0)
        nc.gpsimd.memset(mtrail2[:], 1.0)
        nc.gpsimd.affine_select(
            mlead2[:], mlead2[:], pattern=[[0, 2], [1, 128]], base=0,
            channel_multiplier=-1, compare_op=GE, fill=0.0,
        )
        nc.gpsimd.affine_select(
            mtrail2[:], mtrail2[:], pattern=[[0, 2], [-1, 128]], base=0,
            channel_multiplier=1, compare_op=GE, fill=0.0,
        )
        MASKS = {"mlead": mlead2, "mtrail": mtrail2}

        bpat = msk_pool.tile([128, 1280 - XA], f32, tag="bpat", name="bpat")
        nc.gpsimd.memset(bpat[:], B_VALID)
        for r0, r1, mname in UNI_REGIONS:
            s0 = max(r0, XA)
            if s0 >= r1:
                continue
            w = r1 - s0
            shift = s0 - r0
            if mname == "mlead":  # keep (j_local + shift) - p >= 0
                nc.gpsimd.affine_select(
                    bpat[:, s0 - XA : r1 - XA], bpat[:, s0 - XA : r1 - XA],
                    pattern=[[1, w]], base=shift,
                    channel_multiplier=-1, compare_op=GE, fill=C_MASK,
                )
            else:  # keep p - (j_local + shift) >= 0
                nc.gpsimd.affine_select(
                    bpat[:, s0 - XA : r1 - XA], bpat[:, s0 - XA : r1 - XA],
                    pattern=[[-1, w]], base=-shift,
                    channel_multiplier=1, compare_op=GE, fill=C_MASK,
                )

        # late-needed inputs on the SWDGE queue after mask gen
        nc.gpsimd.dma_start(out=kt_sb[0][:, 768:N], in_=kt_d[0][:, 768:N])
        nc.gpsimd.dma_start(out=vx_sb[1][:], in_=vx_d[1])
        nc.gpsimd.dma_start(out=vx_sb[2][:], in_=vx_d[2])

        # ---- per-(head, phase) emitters ----
        pt_tiles = {}

        def emit_qk(u, h, pi):
            ring = rings[u % 2]
            for kc, off, w in phase_cols(pi, h):
                klhs = kh(h)[:, 128 * kc : 128 * kc + 128]
                qs = chunk_qs(kc)
                a = 0
                while a < w:  # split at ring bank boundaries
                    b = min(((off + a) // 512 + 1) * 512 - off, w)
                    nc.tensor.matmul(
                        ring[:, off + a : off + b],
                        klhs,
                        qh(h)[:, qs + a : qs + b],
                        start=True, stop=True, skip_group_check=True,
                    )
                    a = b

        def emit_exp(u, h, pi):
            ring = rings[u % 2]
            cols = phase_cols(pi, h)
            wtot = cols[-1][1] + cols[-1][2]
            if phase_uniform(pi):
                pa = pt_pool.tile([128, XA], bf16, tag="pt", name=f"pt_h{h}p{pi}")
                nc.scalar.activation(pa[:, :], ring[:, 0:XA], EXP)
                pd = pti_pool.tile(
                    [128, 1280 - XA], i32, tag="pti", name=f"pti_h{h}p{pi}"
                )
                nc.vector.scalar_tensor_tensor(
                    out=pd[:], in0=ring[:, XA:1280], scalar=A_CONST,
                    in1=bpat[:], op0=MULT, op1=ADD,
                )
                pt_tiles[(h, pi)] = (pa, pd)
            else:
                pa = pt_pool.tile([128, wtot], bf16, tag="pt", name=f"pt_h{h}p{pi}")
                nc.scalar.activation(pa[:, 0:wtot], ring[:, 0:wtot], EXP)
                pt_tiles[(h, pi)] = (pa, None)

        def ptslice(h, pi, kc, c0, c1):
            """bf16 AP for chunk-local cols [c0,c1) of chunk kc in phase pi."""
            pa, pd = pt_tiles[(h, pi)]
            off = dict((k, o) for k, o, _ in phase_cols(pi, h))[kc]
            a0, a1 = off + c0, off + c1
            if pd is None or a1 <= XA:
                return pa[:, a0:a1]
            assert a0 >= XA, f"piece straddles XA: {a0}..{a1}"
            return (
                pd[:, a0 - XA : a1 - XA]
                .bitcast(bf16)
                .rearrange("p (n two) -> p n two", two=2)[:, :, 1]
            )

        def mask_eng():
            return nc.gpsimd if MASK_ENG == "g" else nc.vector

        def emit_masks(h, pi):
            pa, pd = pt_tiles[(h, pi)]
            if phase_uniform(pi):
                for r0, r1, mname in UNI_REGIONS:
                    e = min(r1, XA)
                    if r0 >= e:
                        continue
                    m = MASKS[mname]
                    mask_eng().tensor_mul(
                        pa[:, r0:e], pa[:, r0:e], m[:, 0, 0 : e - r0]
                    )
                return
            for kc, off, w in phase_cols(pi, h):
                for c0, c1, mname in chunk_masks(kc):
                    m = MASKS[mname]
                    mask_eng().tensor_mul(
                        pa[:, off + c0 : off + c1],
                        pa[:, off + c0 : off + c1],
                        m[:, 0, 0 : c1 - c0],
                    )

        bank_tiles = {}
        otq = [nc.sync]

        def emit_pv(h, pi):
            for kc, a, b, full in PV_SCHED[pi]:
                cpi = kc // 2  # the chunk's own phase (deferred pieces: < pi)
                bank = a // 512
                key = (h, bank)
                if full:
                    assert key not in bank_tiles
                    bank_tiles[key] = po_pool.tile(
                        [65, 512], f32, tag="ot", name=f"ot_h{h}b{bank}"
                    )
                bt = bank_tiles[key]
                vstat = vx_sb[h][:, kc, :]
                qs = chunk_qs(kc)
                # split at the XA boundary inside uniform phases so each
                # matmul's pt slice is wholly ACT-side or DVE-side
                splits = [(a, b)]
                if phase_uniform(cpi):
                    off = dict((k, o) for k, o, _ in phase_cols(cpi, h))[kc]
                    qx = qs + (XA - off)  # q at the XA boundary
                    if a < qx < b:
                        splits = [(a, qx), (qx, b)]
                for s0, s1 in splits:
                    nc.tensor.matmul(
                        bt[:, s0 - 512 * bank : s1 - 512 * bank],
                        vstat,
                        ptslice(h, cpi, kc, s0 - qs, s1 - qs),
                        start=full and s0 == a, stop=False,
                        skip_group_check=True,
                    )
            done = [bank for bank, dpi in bank_done_for(h).items() if dpi == pi]
            if done:
                bank = done[0]
                bt = bank_tiles.pop((h, bank))
                osb = osb_pool.tile(
                    [65, 512], f32, tag="osb", name=f"osb_h{h}b{bank}"
                )
                # split the copy across ACT and DVE so neither engine's
                # in-order queue inserts a long op ahead of the ring-critical
                # activation / Schraudolph instructions
                nc.scalar.copy(out=osb[:, 0:256], in_=bt[:, 0:256])
                nc.vector.tensor_copy(out=osb[:, 256:512], in_=bt[:, 256:512])
                q = otq[(4 * h + bank + h) % len(otq)]
                q.dma_start(
                    out=ot_d[h][:, 512 * bank : 512 * bank + 512], in_=osb[:]
                )

        # ---- software-pipelined emission: PV lags QK by PV_LAG units ----
        units = [(h, pi) for h in range(HPC) for pi in range(8)]
        L = len(units)
        for u, (h, pi) in enumerate(units):
            emit_qk(u, h, pi)
            emit_exp(u, h, pi)
            emit_masks(h, pi)
            if u >= PV_LAG:
                emit_pv(*units[u - PV_LAG])
            if u >= PV_LAG + 1:
                pt_tiles.pop(units[u - PV_LAG - 1])
        for u in range(L - PV_LAG, L):
            emit_pv(*units[u])
            if u >= 1:
                pt_tiles.pop(units[u - 1])
        pt_tiles.pop(units[L - 1])

    nc.compile()
    _CACHED_NC = nc
    return nc


# ---------------------------------------------------------------------------
# Entry points
# ---------------------------------------------------------------------------


def run(inputs, trace=False, trace_kwargs=None):
    """Returns (output [B,H,N,D] f32, BassKernelResults)."""
    from concourse import bass_utils

    Q = np.asarray(inputs["Q"], np.float32).reshape(B * H, N, D)
    K = np.asarray(inputs["K"], np.float32).reshape(B * H, N, D)
    V = np.asarray(inputs["V"], np.float32).reshape(B * H, N, D)
    in_maps = [prep_core_inputs(Q, K, V, c) for c in range(NCORES)]
    nc = build_module()
    res = bass_utils.run_bass_kernel_spmd(
        nc,
        in_maps,
        core_ids=list(range(NCORES)),
        trace=trace,
        **(trace_kwargs or {}),
    )
    ot_all = [res.results[c]["ot"] for c in range(NCORES)]
    og = host_global_rows(Q, K, V)
    gnum, gden = host_glob_strips(Q, K, V)
    return unprep_output(ot_all, og, gnum, gden), res


def kernel(**inputs) -> np.ndarray:
    out, _ = run(inputs, trace=False)
    return out


# revision 19
# speedup vs baseline: 1.1821x; 1.1353x over previous
"""Longformer attention Bass kernel for 8 TRN2 NeuronCores (v4).

Problem: B=2, H=16, N=2048, D=64, window=256, global positions 0..3.
Sharding: B*H = 32 heads -> 4 heads per core (head-parallel).

v4 changes over v3 (48.5us):
  - Zero-init PV matmuls removed: PV pieces are rescheduled so each O^T
    PSUM bank's FIRST writer is the full-bank piece of chunk 4k+1
    (start=True zeroes the whole 2KB bank); earlier chunks' pieces for
    that bank are deferred into that phase.  -3.4us of PE.
  - exp split between ACT and DVE: per uniform phase the ACT engine
    exps ring[:, 0:XA]; the DVE computes ring[:, XA:1280] with a fused
    Schraudolph bitcast-exp: one scalar_tensor_tensor
      i32 = int32(s * A + Bpat)   (store-convert does the 2^x trick)
    where Bpat is B_VALID on in-band entries and C_MASK on masked ones
    (masked entries land at ~2^-100: the band mask rides for free), and
    the bf16 P^T for the PV matmul is the upper 16 bits of each int32
    read through a stride-2 bitcast view (bf16 round via +2^15 in B).
    Schraudolph rel err ~1.8% RMS on the offloaded columns.
  - mask multiplies for the ACT part moved to GPSIMD (Pool), freeing
    the DVE; dead mc0 mask removed.
  - O^T bank copies split ACT/DVE by knob.
"""

import numpy as np
import ml_dtypes

B, H, N, D = 2, 16, 2048, 64
W = 256
NG = 4  # global positions 0..3
NCORES = 8
HPC = (B * H) // NCORES  # heads per core = 4
NKC = N // 128  # key chunks = 16
BF16 = ml_dtypes.bfloat16

# ---- tuning knobs ----------------------------------------------------------
XA = 512  # uniform-phase cols [0:XA) exp'd on ACT; [XA:1280) Schraudolph on DVE
# NOTE: XA must be 512 — the score ring is physically split into a 1-bank
# "lo" tile (ACT-read) and a 2-bank "hi" tile (DVE-read) so the two exp
# consumers have independent dependency tracking (a shared tile serializes
# its readers in the tile framework, which was the v4b pipeline bottleneck)
SCHR_DELTA = 0.0575  # Schraudolph centering (fraction of 2^23)
A_CONST = float(np.float32((1 << 23) / np.log(2.0)))
B_VALID = float(np.float32((127 << 23) + 32768 - SCHR_DELTA * (1 << 23)))
C_MASK = float(np.float32(1 << 27))
# engine for bf16 mask multiplies on the ACT part: 'g' (Pool) or 'v' (DVE)
MASK_ENG = "g"
# (h, bank) pairs whose O^T copy runs on ACT instead of DVE: spread them so
# neither engine's in-order queue delays the ring-WAR critical chain
COPY_ON_ACT = {(0, 1), (1, 0), (1, 3), (2, 2), (3, 1), (3, 2)}
# PV emission lag in units: larger lag gives the QK->exp->mask->PV chain
# more pipeline slack before it gates the in-order PE program
PV_LAG = 3

# phase grouping of key chunks
PHASES = [[0, 1], [2, 3], [4, 5], [6, 7], [8, 9], [10, 11], [12, 13], [14, 15]]


def phases_for(h):
    return PHASES


# ot bank b is complete after the PV pass of this phase index
BANK_DONE_PHASE = {0: 2, 1: 4, 2: 6, 3: 7}


def bank_done_for(h):
    return BANK_DONE_PHASE


def chunk_qs(kc: int) -> int:
    if kc == 0:
        return 0
    return min(max(128 * kc - W, 0), N - 384)


def chunk_width(kc: int) -> int:
    if kc in (1, 14):
        return 512
    if kc in (0, 15):
        return 384
    return 640


def chunk_masks(kc: int):
    """Mask ops for chunk kc in chunk-local columns: (col0, col1, mask)."""
    if kc == 0:
        return [(256, 384, "mtrail")]
    if kc == 1:
        return [(384, 512, "mtrail")]
    if kc in (14, 15):
        return [(0, 128, "mlead")]
    return [(0, 128, "mlead"), (512, 640, "mtrail")]


def pv_pieces(kc: int):
    """PV output piece spans for chunk kc: (abs_col0, abs_col1)."""
    qs, w = chunk_qs(kc), chunk_width(kc)
    pieces = []
    a = qs
    while a < qs + w:
        b = min((a // 512 + 1) * 512, qs + w)
        pieces.append((a, b))
        a = b
    return pieces


def pv_schedule():
    """{phase: [(kc, a, b, is_full_bank_start)]} with each bank's first
    writer being the full-bank piece of chunk 4k+1 (start=True zeroes the
    bank); earlier chunks' pieces for that bank are deferred to that phase."""
    sched = {pi: [] for pi in range(8)}
    for kc in range(NKC):
        cp = kc // 2
        for a, b in pv_pieces(kc):
            bank = a // 512
            full = kc == 4 * bank + 1 and a == 512 * bank and b == a + 512
            p = cp if kc >= 4 * bank + 1 else 2 * bank
            sched[p].append((kc, a, b, full))
    for p in sched:
        sched[p].sort(key=lambda t: (not t[3], t[0], t[1]))
    return sched


PV_SCHED = pv_schedule()

# uniform-phase mask regions in absolute phase columns
UNI_REGIONS = [(0, 128, "mlead"), (512, 640, "mtrail"),
               (640, 768, "mlead"), (1152, 1280, "mtrail")]


def phase_cols(pi, h=0):
    """[(kc, col_off, width)] within the phase tile."""
    off = 0
    out = []
    for kc in phases_for(h)[pi]:
        w = chunk_width(kc)
        out.append((kc, off, w))
        off += w
    return out


def phase_uniform(pi):
    return all(chunk_width(kc) == 640 for kc in PHASES[pi])


# ---------------------------------------------------------------------------
# Numpy model of the exact device algorithm (geometry validation)
# ---------------------------------------------------------------------------


def _mask_tiles_np():
    p = np.arange(128)[:, None]
    j = np.arange(128)[None, :]
    return {
        "mlead": (j >= p).astype(np.float32),
        "mtrail": (j <= p).astype(np.float32),
    }


def _bpat_np():
    """[128, 1280-XA] f32 Schraudolph bias pattern for uniform phases."""
    bp = np.full((128, 1280 - XA), np.float32(B_VALID), np.float32)
    masks = _mask_tiles_np()
    for r0, r1, mname in UNI_REGIONS:
        s0, s1 = max(r0, XA), r1
        if s0 >= s1:
            continue
        p = np.arange(128)[:, None]
        j = np.arange(s0 - r0, s1 - r0)[None, :]
        keep = (j >= p) if mname == "mlead" else (j <= p)
        bp[:, s0 - XA : s1 - XA] = np.where(keep, np.float32(B_VALID),
                                            np.float32(C_MASK))
    return bp


def _schraudolph_np(st, bpat):
    y = st.astype(np.float32) * np.float32(A_CONST) + bpat
    i = y.astype(np.int32)
    return ((i.view(np.uint32) >> np.uint32(16)).astype(np.uint16)
            .view(BF16).astype(np.float32))


def numpy_model_head(qT, kT, vx):
    """qT/kT: [64, N] bf16-rounded f32 (q pre-scaled); vx: [N, 65] bf16-rounded.

    Returns OT [65, N] f32 (unnormalized band-only O^T + denominator row).
    """
    qT = qT.astype(np.float32)
    kT = kT.astype(np.float32)
    vx = vx.astype(np.float32)
    masks = _mask_tiles_np()
    bpat = _bpat_np()
    ot = np.zeros((65, N), np.float32)
    for pi in range(8):
        cols = phase_cols(pi)
        wtot = cols[-1][1] + cols[-1][2]
        st = np.empty((128, wtot), np.float32)
        for kc, off, w in cols:
            qs = chunk_qs(kc)
            kk = slice(128 * kc, 128 * kc + 128)
            st[:, off : off + w] = kT[:, kk].T @ qT[:, qs : qs + w]
        if phase_uniform(pi):
            pt = np.empty((128, wtot), np.float32)
            pt[:, :XA] = np.exp(st[:, :XA]).astype(BF16).astype(np.float32)
            pt[:, XA:] = _schraudolph_np(st[:, XA:], bpat)
            for r0, r1, mname in UNI_REGIONS:
                e = min(r1, XA)
                if r0 < e:
                    pt[:, r0:e] *= masks[mname][:, : e - r0]
        else:
            pt = np.exp(st).astype(BF16).astype(np.float32)
            for kc, off, w in cols:
                for c0, c1, mname in chunk_masks(kc):
                    pt[:, off + c0 : off + c1] *= masks[mname][:, : c1 - c0]
        pt = pt.astype(BF16).astype(np.float32)
        for kc, off, w in cols:
            qs = chunk_qs(kc)
            kk = slice(128 * kc, 128 * kc + 128)
            ot[:, qs : qs + w] += vx[kk].T @ pt[:, off : off + w]
    return ot


# ---------------------------------------------------------------------------
# Host-side prep / unprep
# ---------------------------------------------------------------------------


def prep_core_inputs(Q, K, V, core):
    """Q/K/V: [B*H, N, D] f32. Returns the in_map for one core."""
    h0 = core * HPC
    qt = np.empty((2, 128, N), BF16)
    kt = np.empty((2, 128, N), BF16)
    vx = np.zeros((HPC, 128, NKC, 65), BF16)
    for p in range(2):
        for s in range(2):
            h = h0 + 2 * p + s
            qt[p, 64 * s : 64 * s + 64] = (Q[h].T * np.float32(0.125)).astype(BF16)
            kt[p, 64 * s : 64 * s + 64] = K[h].T.astype(BF16)
    for i in range(HPC):
        v = np.concatenate([V[h0 + i], np.ones((N, 1), np.float32)], axis=1)
        vx[i] = v.reshape(NKC, 128, 65).transpose(1, 0, 2).astype(BF16)
    return {"qt": qt, "kt": kt, "vx": vx}


def host_glob_strips(Q, K, V):
    """f32 contributions of the 4 global KEYS beyond the window (k < q-256)."""
    scale = np.float32(0.125)
    s = np.einsum("hqd,hkd->hqk", Q[:, 256:].astype(np.float32), K[:, 0:NG]) * scale
    e = np.exp(s)  # [BH, N-256, NG]
    q_abs = np.arange(256, N)[None, :, None]
    k_idx = np.arange(NG)[None, None, :]
    e = e * (k_idx < q_abs - 256)
    gnum = np.einsum("hqk,hkd->hqd", e, V[:, 0:NG])
    gden = e.sum(axis=-1)
    return gnum, gden


def host_global_rows(Q, K, V):
    """Exact f32 attention for the 4 global query rows of every head."""
    scale = np.float32(1.0 / np.sqrt(D))
    s = np.einsum("hqd,hkd->hqk", Q[:, :NG].astype(np.float32), K) * scale
    s -= s.max(axis=-1, keepdims=True)
    p = np.exp(s)
    p /= p.sum(axis=-1, keepdims=True)
    return np.einsum("hqk,hkd->hqd", p, V)


def unprep_output(ot_all, og, gnum, gden):
    out = np.empty((B * H, N, D), np.float32)
    for core in range(NCORES):
        ot = np.asarray(ot_all[core])
        for i in range(HPC):
            h = core * HPC + i
            num = ot[i, :D, :].T.copy()  # [N, D]
            den = ot[i, D, :].copy()  # [N]
            num[256:] += gnum[h]
            den[256:] += gden[h]
            out[h] = num / den[:, None]
    out[:, 0:NG] = og
    return out.reshape(B, H, N, D)


# ---------------------------------------------------------------------------
# Bass module
# ---------------------------------------------------------------------------

_CACHED_NC = None


def build_module():
    global _CACHED_NC
    if _CACHED_NC is not None:
        return _CACHED_NC
    from contextlib import ExitStack

    import concourse.bass as bass  # noqa: F401
    import concourse.tile as tile
    from concourse import bacc, mybir

    f32 = mybir.dt.float32
    bf16 = mybir.dt.bfloat16
    i32 = mybir.dt.int32
    EXP = mybir.ActivationFunctionType.Exp
    GE = mybir.AluOpType.is_ge
    MULT = mybir.AluOpType.mult
    ADD = mybir.AluOpType.add

    nc = bacc.Bacc("TRN2", target_bir_lowering=False, debug=False)
    qt_d = nc.dram_tensor("qt", [2, 128, N], bf16, kind="ExternalInput")
    kt_d = nc.dram_tensor("kt", [2, 128, N], bf16, kind="ExternalInput")
    vx_d = nc.dram_tensor("vx", [HPC, 128, NKC, 65], bf16, kind="ExternalInput")
    ot_d = nc.dram_tensor("ot", [HPC, 65, N], f32, kind="ExternalOutput")

    with tile.TileContext(nc) as tc, ExitStack() as ctx:
        io_pool = ctx.enter_context(tc.tile_pool(name="io", bufs=1))
        msk_pool = ctx.enter_context(tc.tile_pool(name="msk", bufs=1))
        pt_pool = ctx.enter_context(tc.tile_pool(name="ptp", bufs=PV_LAG + 4))
        pti_pool = ctx.enter_context(tc.tile_pool(name="pti", bufs=PV_LAG + 4))
        osb_pool = ctx.enter_context(tc.tile_pool(name="osb", bufs=3))
        ring_pool = ctx.enter_context(tc.tile_pool(name="ring", bufs=1, space="PSUM"))
        po_pool = ctx.enter_context(tc.tile_pool(name="po", bufs=2, space="PSUM"))

        # ---- static PSUM: two score rings, each split lo (1 bank, ACT) +
        # hi (2 banks, DVE) so the two exp readers don't serialize ----
        rings = []
        for rn in ("A", "B"):
            lo = ring_pool.tile([128, 512], f32, tag=f"rl{rn}", name=f"ringlo{rn}")
            hi = ring_pool.tile([128, 1024], f32, tag=f"rh{rn}", name=f"ringhi{rn}")
            rings.append((lo, hi))

        # ---- inputs ----
        qt_sb = []
        kt_sb = []
        vx_sb = []
        for pair in range(2):
            qtp = io_pool.tile([128, N], bf16, tag=f"qt{pair}", name=f"qt{pair}")
            ktp = io_pool.tile([128, N], bf16, tag=f"kt{pair}", name=f"kt{pair}")
            qt_sb.append(qtp)
            kt_sb.append(ktp)
        for h in range(HPC):
            vxh = io_pool.tile([128, NKC, 65], bf16, tag=f"vx{h}", name=f"vx{h}")
            vx_sb.append(vxh)
        # issue order == transfer order; tiny pair-0 lead pieces unblock the
        # first QK phase as early as possible, everything else streams
        nc.sync.dma_start(out=kt_sb[0][:, 0:256], in_=kt_d[0][:, 0:256])
        nc.scalar.dma_start(out=qt_sb[0][:, 0:512], in_=qt_d[0][:, 0:512])
        nc.sync.dma_start(out=kt_sb[0][:, 256:768], in_=kt_d[0][:, 256:768])
        nc.scalar.dma_start(out=qt_sb[0][:, 512:1152], in_=qt_d[0][:, 512:1152])
        nc.sync.dma_start(out=qt_sb[0][:, 1152:N], in_=qt_d[0][:, 1152:N])
        nc.scalar.dma_start(out=vx_sb[0][:], in_=vx_d[0])
        nc.sync.dma_start(out=kt_sb[1][:], in_=kt_d[1])
        nc.scalar.dma_start(out=qt_sb[1][:], in_=qt_d[1])
        nc.sync.dma_start(out=vx_sb[3][:], in_=vx_d[3])

        # ---- warm the PE pstate while the first DMAs land ----
        wu = msk_pool.tile([64, 512], bf16, tag="wu", name="wu")
        nc.vector.memset(wu[:, 0:128], 0.0)
        nc.tensor.matmul(
            rings[1][0][:, 0:128], wu[:, 0:128], wu[:, 0:128],
            start=True, stop=True, skip_group_check=True,
        )
        nc.vector.memset(wu[:, 128:512], 0.0)
        for i in range(5):
            nc.tensor.matmul(
                rings[1][0][:, 0:512], wu[:, 0:128], wu[:],
                start=True, stop=True, skip_group_check=True,
            )

        def qh(h):
            return qt_sb[h // 2][64 * (h % 2) : 64 * (h % 2) + 64, :]

        def kh(h):
            return kt_sb[h // 2][64 * (h % 2) : 64 * (h % 2) + 64, :]

        # ---- mask tiles (0/1 bf16) + Schraudolph bias pattern (f32) ----
        mlead2 = msk_pool.tile([128, 2, 128], bf16, tag="mlead2", name="mlead2")
        mtrail2 = msk_pool.tile([128, 2, 128], bf16, tag="mtrail2", name="mtrail2")
        nc.gpsimd.memset(mlead2[:], 1.0)
        nc.gpsimd.memset(mtrail2[:], 1.0)
        nc.gpsimd.affine_select(
            mlead2[:], mlead2[:], pattern=[[0, 2], [1, 128]], base=0,
            channel_multiplier=-1, compare_op=GE, fill=0.0,
        )
        nc.gpsimd.affine_select(
            mtrail2[:], mtrail2[:], pattern=[[0, 2], [-1, 128]], base=0,
            channel_multiplier=1, compare_op=GE, fill=0.0,
        )
        MASKS = {"mlead": mlead2, "mtrail": mtrail2}

        bpat = msk_pool.tile([128, 1280 - XA], f32, tag="bpat", name="bpat")
        nc.gpsimd.memset(bpat[:], B_VALID)
        for r0, r1, mname in UNI_REGIONS:
            s0 = max(r0, XA)
            if s0 >= r1:
                continue
            w = r1 - s0
            shift = s0 - r0
            if mname == "mlead":  # keep (j_local + shift) - p >= 0
                nc.gpsimd.affine_select(
                    bpat[:, s0 - XA : r1 - XA], bpat[:, s0 - XA : r1 - XA],
                    pattern=[[1, w]], base=shift,
                    channel_multiplier=-1, compare_op=GE, fill=C_MASK,
                )
            else:  # keep p - (j_local + shift) >= 0
                nc.gpsimd.affine_select(
                    bpat[:, s0 - XA : r1 - XA], bpat[:, s0 - XA : r1 - XA],
                    pattern=[[-1, w]], base=-shift,
                    channel_multiplier=1, compare_op=GE, fill=C_MASK,
                )

        # late-needed inputs on the SWDGE queue after mask gen
        nc.gpsimd.dma_start(out=kt_sb[0][:, 768:N], in_=kt_d[0][:, 768:N])
        nc.gpsimd.dma_start(out=vx_sb[1][:], in_=vx_d[1])
        nc.gpsimd.dma_start(out=vx_sb[2][:], in_=vx_d[2])

        # ---- per-(head, phase) emitters ----
        pt_tiles = {}

        def ring_dst(u, c0, c1):
            """AP in the split ring for phase cols [c0, c1) (no 512-crossing)."""
            lo, hi = rings[u % 2]
            if c1 <= 512:
                return lo[:, c0:c1]
            assert c0 >= 512
            return hi[:, c0 - 512 : c1 - 512]

        def emit_qk(u, h, pi):
            for kc, off, w in phase_cols(pi, h):
                klhs = kh(h)[:, 128 * kc : 128 * kc + 128]
                qs = chunk_qs(kc)
                a = 0
                while a < w:  # split at ring bank boundaries
                    b = min(((off + a) // 512 + 1) * 512 - off, w)
                    nc.tensor.matmul(
                        ring_dst(u, off + a, off + b),
                        klhs,
                        qh(h)[:, qs + a : qs + b],
                        start=True, stop=True, skip_group_check=True,
                    )
                    a = b

        def emit_exp(u, h, pi):
            lo, hi = rings[u % 2]
            cols = phase_cols(pi, h)
            wtot = cols[-1][1] + cols[-1][2]
            if phase_uniform(pi):
                pa = pt_pool.tile([128, XA], bf16, tag="pt", name=f"pt_h{h}p{pi}")
                pd = pti_pool.tile(
                    [128, 1280 - XA], i32, tag="pti", name=f"pti_h{h}p{pi}"
                )
                nc.vector.scalar_tensor_tensor(
                    out=pd[:], in0=hi[:, 0 : 1280 - XA], scalar=A_CONST,
                    in1=bpat[:], op0=MULT, op1=ADD,
                )
                nc.scalar.activation(pa[:, :], lo[:, 0:XA], EXP)
                pt_tiles[(h, pi)] = (pa, pd)
            else:
                pa = pt_pool.tile([128, wtot], bf16, tag="pt", name=f"pt_h{h}p{pi}")
                nc.scalar.activation(pa[:, 0:512], lo[:, 0:512], EXP)
                nc.scalar.activation(pa[:, 512:wtot], hi[:, 0 : wtot - 512], EXP)
                pt_tiles[(h, pi)] = (pa, None)

        def ptslice(h, pi, kc, c0, c1):
            """bf16 AP for chunk-local cols [c0,c1) of chunk kc in phase pi."""
            pa, pd = pt_tiles[(h, pi)]
            off = dict((k, o) for k, o, _ in phase_cols(pi, h))[kc]
            a0, a1 = off + c0, off + c1
            if pd is None or a1 <= XA:
                return pa[:, a0:a1]
            assert a0 >= XA, f"piece straddles XA: {a0}..{a1}"
            return (
                pd[:, a0 - XA : a1 - XA]
                .bitcast(bf16)
                .rearrange("p (n two) -> p n two", two=2)[:, :, 1]
            )

        def mask_eng():
            return nc.gpsimd if MASK_ENG == "g" else nc.vector

        def emit_masks(h, pi):
            pa, pd = pt_tiles[(h, pi)]
            if phase_uniform(pi):
                for r0, r1, mname in UNI_REGIONS:
                    e = min(r1, XA)
                    if r0 >= e:
                        continue
                    m = MASKS[mname]
                    mask_eng().tensor_mul(
                        pa[:, r0:e], pa[:, r0:e], m[:, 0, 0 : e - r0]
                    )
                return
            for kc, off, w in phase_cols(pi, h):
                for c0, c1, mname in chunk_masks(kc):
                    m = MASKS[mname]
                    mask_eng().tensor_mul(
                        pa[:, off + c0 : off + c1],
                        pa[:, off + c0 : off + c1],
                        m[:, 0, 0 : c1 - c0],
                    )

        bank_tiles = {}
        otq = [nc.sync]

        def emit_pv(h, pi):
            for kc, a, b, full in PV_SCHED[pi]:
                cpi = kc // 2  # the chunk's own phase (deferred pieces: < pi)
                bank = a // 512
                key = (h, bank)
                if full:
                    assert key not in bank_tiles
                    bank_tiles[key] = po_pool.tile(
                        [65, 512], f32, tag="ot", name=f"ot_h{h}b{bank}"
                    )
                bt = bank_tiles[key]
                vstat = vx_sb[h][:, kc, :]
                qs = chunk_qs(kc)
                # split at the XA boundary inside uniform phases so each
                # matmul's pt slice is wholly ACT-side or DVE-side
                splits = [(a, b)]
                if phase_uniform(cpi):
                    off = dict((k, o) for k, o, _ in phase_cols(cpi, h))[kc]
                    qx = qs + (XA - off)  # q at the XA boundary
                    if a < qx < b:
                        splits = [(a, qx), (qx, b)]
                for s0, s1 in splits:
                    nc.tensor.matmul(
                        bt[:, s0 - 512 * bank : s1 - 512 * bank],
                        vstat,
                        ptslice(h, cpi, kc, s0 - qs, s1 - qs),
                        start=full and s0 == a, stop=False,
                        skip_group_check=True,
                    )
            done = [bank for bank, dpi in bank_done_for(h).items() if dpi == pi]
            if done:
                bank = done[0]
                bt = bank_tiles.pop((h, bank))
                osb = osb_pool.tile(
                    [65, 512], f32, tag="osb", name=f"osb_h{h}b{bank}"
                )
                # split the copy across ACT and DVE so neither engine's
                # in-order queue inserts a long op ahead of the ring-critical
                # activation / Schraudolph instructions
                nc.scalar.copy(out=osb[:, 0:256], in_=bt[:, 0:256])
                nc.vector.tensor_copy(out=osb[:, 256:512], in_=bt[:, 256:512])
                q = otq[(4 * h + bank + h) % len(otq)]
                q.dma_start(
                    out=ot_d[h][:, 512 * bank : 512 * bank + 512], in_=osb[:]
                )

        # ---- software-pipelined emission: PV lags QK by PV_LAG units ----
        units = [(h, pi) for h in range(HPC) for pi in range(8)]
        L = len(units)
        for u, (h, pi) in enumerate(units):
            emit_qk(u, h, pi)
            emit_exp(u, h, pi)
            emit_masks(h, pi)
            if u >= PV_LAG:
                emit_pv(*units[u - PV_LAG])
            if u >= PV_LAG + 1:
                pt_tiles.pop(units[u - PV_LAG - 1])
        for u in range(L - PV_LAG, L):
            emit_pv(*units[u])
            if u >= 1:
                pt_tiles.pop(units[u - 1])
        pt_tiles.pop(units[L - 1])

    nc.compile()
    _CACHED_NC = nc
    return nc


# ---------------------------------------------------------------------------
# Entry points
# ---------------------------------------------------------------------------


def run(inputs, trace=False, trace_kwargs=None):
    """Returns (output [B,H,N,D] f32, BassKernelResults)."""
    from concourse import bass_utils

    Q = np.asarray(inputs["Q"], np.float32).reshape(B * H, N, D)
    K = np.asarray(inputs["K"], np.float32).reshape(B * H, N, D)
    V = np.asarray(inputs["V"], np.float32).reshape(B * H, N, D)
    in_maps = [prep_core_inputs(Q, K, V, c) for c in range(NCORES)]
    nc = build_module()
    res = bass_utils.run_bass_kernel_spmd(
        nc,
        in_maps,
        core_ids=list(range(NCORES)),
        trace=trace,
        **(trace_kwargs or {}),
    )
    ot_all = [res.results[c]["ot"] for c in range(NCORES)]
    og = host_global_rows(Q, K, V)
    gnum, gden = host_glob_strips(Q, K, V)
    return unprep_output(ot_all, og, gnum, gden), res


def kernel(**inputs) -> np.ndarray:
    out, _ = run(inputs, trace=False)
    return out
